# revision 24
# baseline (speedup 1.0000x reference)
"""HadamardAttention Trainium2 kernel — 8-core data-parallel over batch.

Per core (one batch element b), everything in "transposed" activation
layout [C on partitions, N on free dim]:

  phase A: qT/kT projections -> Hadamard product -> per-head reduction
           (selector matmul, SCALE folded in) -> tiny MLP -> masked
           scores awFull [H, N] (fp32)
  phase B: softmax over N (free dim) on [32, 4096]
  phase C: vT projection (x re-streamed), head-broadcast of weights
           (selector matmul), p2T = aw*vT, final out-projection which
           naturally restores natural [N, C] layout (p2T is the lhsT).

Host-side prep is layout-only (transpose/reshape) plus dtype casts to
bf16 for TensorE operands; all FLOPs happen on device.
"""
import sys

if "/opt/trn_rl_repo" not in sys.path:
    sys.path.insert(0, "/opt/trn_rl_repo")

import numpy as np
import ml_dtypes
from contextlib import ExitStack

import concourse.bass as bass
import concourse.bacc as bacc
import concourse.tile as tile
from concourse import mybir
from concourse.bass_utils import run_bass_kernel_spmd

# antenv.axon_hooks is absent in some images; shim it so trace=True can
# reach the NTFF profiler. Harmless no-op for trace=False runs.
try:
    from antenv.axon_hooks import get_axon_ntff_profile_hook  # noqa: F401
except ImportError:
    try:
        import types
        import antenv

        _hooks = types.ModuleType("antenv.axon_hooks")
        _hooks._hook = None
        _hooks.set_axon_ntff_profile_hook = lambda h: setattr(_hooks, "_hook", h)
        _hooks.get_axon_ntff_profile_hook = lambda: _hooks._hook
        sys.modules["antenv.axon_hooks"] = _hooks
        antenv.axon_hooks = _hooks
        from trn_agent_boot.trn_boot import _ntff_profile_via_ctypes

        _hooks.set_axon_ntff_profile_hook(
            _ntff_profile_via_ctypes("/opt/axon/libaxon_pjrt.so"))
    except Exception:
        pass

B, N, C, H, D = 8, 4096, 1024, 32, 32
SCALE = float(D) ** -0.5
P = 128
CK = C // P          # 8 chunks of the channel dim
NQ = 4               # token quarters
TQ = N // NQ         # 1024 tokens per quarter
TC = 512             # moving free dim per matmul
BF16 = mybir.dt.bfloat16
F32 = mybir.dt.float32
I32 = mybir.dt.int32
AF = mybir.ActivationFunctionType
ALU = mybir.AluOpType


def _build(use_mask):
    nc = bacc.Bacc("TRN2", num_devices=8)

    xTr = nc.declare_dram_parameter("xTr", [P, CK, N], BF16, isOutput=False)
    if use_mask:
        mask = nc.declare_dram_parameter("mask", [1, N], I32, isOutput=False)
    wq = nc.declare_dram_parameter("wq", [P, CK, C], BF16, isOutput=False)
    wk = nc.declare_dram_parameter("wk", [P, CK, C], BF16, isOutput=False)
    wv = nc.declare_dram_parameter("wv", [P, CK, C], BF16, isOutput=False)
    wo = nc.declare_dram_parameter("wo", [P, CK, C], BF16, isOutput=False)
    # w1x[c, j] = SCALE * W1[head(c), j]: folds the per-head reduction
    # (sel1) and the MLP first layer into one PE accumulation chain.
    w1x = nc.declare_dram_parameter("w1x", [P, CK, 2 * D], BF16, isOutput=False)
    w2 = nc.declare_dram_parameter("w2", [2 * D, H], BF16, isOutput=False)
    # fp32 head-selector for the tiny s4 rescale-expansion matmuls
    sel2 = nc.declare_dram_parameter("sel2", [H, CK, P], F32, isOutput=False)
    bq = nc.declare_dram_parameter("bq", [P, CK], F32, isOutput=False)
    bk = nc.declare_dram_parameter("bk", [P, CK], F32, isOutput=False)
    bv = nc.declare_dram_parameter("bv", [P, CK], F32, isOutput=False)
    b1 = nc.declare_dram_parameter("b1", [2 * D, 1], F32, isOutput=False)
    b2 = nc.declare_dram_parameter("b2", [H, 1], F32, isOutput=False)
    bo = nc.declare_dram_parameter("bo", [1, C], BF16, isOutput=False)
    ones = nc.declare_dram_parameter("ones", [1, P], BF16, isOutput=False)
    out = nc.declare_dram_parameter("out", [N, C], F32, isOutput=True)

    with tile.TileContext(nc) as tc:
        with ExitStack() as ctx:
            wpool = ctx.enter_context(tc.tile_pool(name="wpool", bufs=1))
            const = ctx.enter_context(tc.tile_pool(name="const", bufs=1))
            narrow = ctx.enter_context(tc.tile_pool(name="narrow", bufs=1))
            small = ctx.enter_context(tc.tile_pool(name="small", bufs=2))
            xin = ctx.enter_context(tc.tile_pool(name="xin", bufs=2))
            ppool = ctx.enter_context(tc.tile_pool(name="ppool", bufs=2))
            qv = ctx.enter_context(tc.tile_pool(name="qv", bufs=4))
            awbp = ctx.enter_context(tc.tile_pool(name="awbp", bufs=8))
            ypool = ctx.enter_context(tc.tile_pool(name="ypool", bufs=4))
            bank = ctx.enter_context(
                tc.tile_pool(name="bank", bufs=6, space="PSUM"))
            sbank = ctx.enter_context(
                tc.tile_pool(name="sbank", bufs=2, space="PSUM"))
            dpool = ctx.enter_context(
                tc.tile_pool(name="dpool", bufs=1, space="DRAM"))

            # ---- constants / weights -------------------------------------
            # per-kc tiles: dependency tracking is per-tile, so chunked
            # tiles let the first matmuls start after 2 DMAs, not 16.
            wq_sb = [wpool.tile([P, C], BF16, tag=f"wq{kc}",
                                name=f"wq{kc}") for kc in range(CK)]
            wk_sb = [wpool.tile([P, C], BF16, tag=f"wk{kc}",
                                name=f"wk{kc}") for kc in range(CK)]
            wv_sb = [wpool.tile([P, C], BF16, tag=f"wv{kc}",
                                name=f"wv{kc}") for kc in range(CK)]
            wo_sb = [wpool.tile([P, C], BF16, tag=f"wo{kc}",
                                name=f"wo{kc}") for kc in range(CK)]
            # startup-critical loads first, interleaved per-kc so the first
            # accumulation group's operands land ASAP: xt(q0) + wq on the
            # sync queue, wk in parallel on the gpsimd queue.
            xt0 = [xin.tile([P, TQ], BF16, tag=f"xin{kc}", name=f"xt0_{kc}")
                   for kc in range(CK)]
            for kc in range(CK):
                nc.sync.dma_start(out=xt0[kc][:], in_=xTr[:, kc, 0:TQ])
                nc.sync.dma_start(out=wq_sb[kc][:], in_=wq[:, kc, :])
                nc.gpsimd.dma_start(out=wk_sb[kc][:], in_=wk[:, kc, :])
            w1x_sb = const.tile([P, CK, 2 * D], BF16, tag="w1x")
            w2_sb = const.tile([2 * D, H], BF16, tag="w2")
            sel2_sb = const.tile([H, CK, P], F32, tag="sel2")
            bq_sb = const.tile([P, CK], F32, tag="bq")
            bk_sb = const.tile([P, CK], F32, tag="bk")
            bv_sb = const.tile([P, CK], F32, tag="bv")
            b1_sb = const.tile([2 * D, 1], F32, tag="b1")
            b2_sb = const.tile([H, 1], F32, tag="b2")
            bo_sb = const.tile([1, C], BF16, tag="bo")
            ones_sb = const.tile([1, P], BF16, tag="ones")
            for t_sb, t_dr in ((w1x_sb, w1x), (w2_sb, w2),
                               (sel2_sb, sel2), (bq_sb, bq), (bk_sb, bk),
                               (bv_sb, bv), (b1_sb, b1), (b2_sb, b2),
                               (bo_sb, bo), (ones_sb, ones)):
                nc.sync.dma_start(out=t_sb[:], in_=t_dr[:])

            if use_mask:
                # additive mask row: 0 where mask==1, -1e9 where mask==0.
                # mask_sb borrows an xin slot (same byte size, bf16 tiles).
                mask_sb = const.tile([1, N], I32, tag="mask")
                nc.sync.dma_start(out=mask_sb[:, :], in_=mask[:, :])
                madd = narrow.tile([1, N], BF16, tag="madd")
                nc.vector.tensor_scalar(
                    out=madd[:], in0=mask_sb[:, :],
                    scalar1=1e9, scalar2=-1e9, op0=ALU.mult, op1=ALU.add)

            # bo replicated across partitions via a step-0 DMA broadcast,
            # so the out-projection bias is a DVE add (not PE matmuls).
            bo_rep = const.tile([P, C], BF16, tag="bo_rep")
            bo_ap = bo[:, :]
            bo_bcast = bass.AP(tensor=bo_ap.tensor, offset=bo_ap.offset,
                               ap=[[0, P], list(bo_ap.ap)[1]])
            nc.gpsimd.dma_start(out=bo_rep[:], in_=bo_bcast)

            awFull = narrow.tile([H, N], F32, tag="awFull")
            awn = narrow.tile([H, N], BF16, tag="awn")
            maxP = narrow.tile([H, 2 * NQ], F32, tag="maxP")
            sumP = narrow.tile([H, 2 * NQ], F32, tag="sumP")
            negmax = narrow.tile([H, 1], F32, tag="negmax")
            sums = narrow.tile([H, 1], F32, tag="sums")
            inv = narrow.tile([H, 1], F32, tag="inv")

            # awn holds the UNNORMALIZED per-chunk exp(s - chunkmax); the
            # flash-softmax correction factor is applied per-partition in
            # phase C. This lets the head-broadcast DMAs (via a DRAM
            # scratch: step-0 partition APs need a DRAM source) run during
            # phase A, off the phase B critical path.
            awn_dr = dpool.tile([H, N], BF16, tag="awn_dr")
            awb_tiles = {}

            # ---- phase A: scores ----------------------------------------
            for iq in range(NQ):
                if iq == 0:
                    xt = xt0
                else:
                    xt = [xin.tile([P, TQ], BF16, tag=f"xin{kc}",
                                   name=f"xtA{iq}_{kc}")
                          for kc in range(CK)]
                    for kc in range(CK):
                        nc.sync.dma_start(
                            out=xt[kc][:],
                            in_=xTr[:, kc, iq * TQ:(iq + 1) * TQ])
                pT = ppool.tile([P, CK, TQ], BF16, tag="pT")
                for mc in range(CK):
                    ps_q = [bank.tile([P, TC], F32, tag="bank",
                                      name=f"psq_{iq}_{mc}_{t}")
                            for t in range(2)]
                    for kc in range(CK):
                        for t in range(2):
                            nc.tensor.matmul(
                                ps_q[t][:],
                                wq_sb[kc][:, mc * P:(mc + 1) * P],
                                xt[kc][:, t * TC:(t + 1) * TC],
                                start=(kc == 0), stop=(kc == CK - 1))
                    q_mc = qv.tile([P, 2, TC], BF16, tag="qv")
                    for t in range(2):
                        nc.scalar.activation(
                            q_mc[:, t, :], ps_q[t][:], AF.Identity,
                            bias=bq_sb[:, mc:mc + 1])
                    ps_k = [bank.tile([P, TC], F32, tag="bank",
                                      name=f"psk_{iq}_{mc}_{t}")
                            for t in range(2)]
                    for kc in range(CK):
                        for t in range(2):
                            nc.tensor.matmul(
                                ps_k[t][:],
                                wk_sb[kc][:, mc * P:(mc + 1) * P],
                                xt[kc][:, t * TC:(t + 1) * TC],
                                start=(kc == 0), stop=(kc == CK - 1))
                    for t in range(2):
                        nc.vector.scalar_tensor_tensor(
                            out=pT[:, mc, t * TC:(t + 1) * TC],
                            in0=ps_k[t][:], scalar=bk_sb[:, mc:mc + 1],
                            in1=q_mc[:, t, :], op0=ALU.add, op1=ALU.mult)
                for t in range(2):
                    j = iq * 2 + t
                    # (sel1 ∘ W1) fused: a1 = W1x^T @ pT directly from pT
                    ps_a1 = sbank.tile([2 * D, TC], F32, tag="sbank")
                    for ci in range(CK):
                        nc.tensor.matmul(
                            ps_a1[:], w1x_sb[:, ci, :],
                            pT[:, ci, t * TC:(t + 1) * TC],
                            start=(ci == 0), stop=(ci == CK - 1))
                    a1 = small.tile([2 * D, TC], BF16, tag="a1")
                    nc.scalar.activation(a1[:], ps_a1[:], AF.Relu,
                                         bias=b1_sb[:])
                    ps_aw2 = sbank.tile([2 * D, TC], F32, tag="sbank")
                    nc.tensor.matmul(ps_aw2[:H, :], w2_sb[:], a1[:],
                                     start=True, stop=not use_mask)
                    if use_mask:
                        nc.tensor.matmul(ps_aw2[:H, :], ones_sb[:1, :H],
                                         madd[:1, j * TC:(j + 1) * TC],
                                         start=False, stop=True)
                    nc.scalar.activation(
                        awFull[:, j * TC:(j + 1) * TC], ps_aw2[:H, :],
                        AF.Identity, bias=b2_sb[:])
                    # flash-style: per-chunk -max, then exp with that max;
                    # the global correction factor is folded into phase C.
                    nc.vector.reduce_max(
                        out=maxP[:, j:j + 1],
                        in_=awFull[:, j * TC:(j + 1) * TC],
                        axis=mybir.AxisListType.X, negate=True)
                    nc.scalar.activation(awn[:, j * TC:(j + 1) * TC],
                                         awFull[:, j * TC:(j + 1) * TC],
                                         AF.Exp, bias=maxP[:, j:j + 1],
                                         accum_out=sumP[:, j:j + 1])
                    nc.gpsimd.dma_start(out=awn_dr[:, j * TC:(j + 1) * TC],
                                        in_=awn[:, j * TC:(j + 1) * TC])
                # head-broadcast for this quarter: awn[h, n] -> awb[c, n]
                # for c in head h, as partition-broadcast DMAs on the
                # (idle) gpsimd queue. awbp pool backpressure paces them.
                for mc in range(CK):
                    awb_mc = awbp.tile([P, TQ], BF16, tag="awb",
                                       name=f"awb_{iq}_{mc}")
                    for h in range(4):
                        src = awn_dr[4 * mc + h:4 * mc + h + 1,
                                     iq * TQ:(iq + 1) * TQ]
                        bc = bass.AP(tensor=src.tensor, offset=src.offset,
                                     ap=[[0, 32]] + list(src.ap)[1:])
                        nc.gpsimd.dma_start(
                            out=awb_mc[h * 32:(h + 1) * 32, :], in_=bc)
                    awb_tiles[(iq, mc)] = awb_mc

            # ---- phase B: flash-softmax correction (tiny) ---------------
            # maxP holds -chunkmax; global negmax = min_j maxP[j].
            nc.vector.tensor_reduce(out=negmax[:], in_=maxP[:],
                                    axis=mybir.AxisListType.X,
                                    op=ALU.min)
            # corr[j] = exp(chunkmax_j - globalmax) = exp(-maxP_j + negmax)
            corr = narrow.tile([H, 2 * NQ], F32, tag="corr")
            nc.scalar.activation(corr[:], maxP[:], AF.Exp,
                                 bias=negmax[:], scale=-1.0)
            wsum = narrow.tile([H, 2 * NQ], F32, tag="wsum")
            nc.vector.tensor_mul(wsum[:], sumP[:], corr[:])
            nc.vector.reduce_sum(out=sums[:], in_=wsum[:],
                                 axis=mybir.AxisListType.X)
            nc.vector.reciprocal(out=inv[:], in_=sums[:])
            sfac = narrow.tile([H, 2 * NQ], F32, tag="sfac")
            nc.vector.tensor_scalar(out=sfac[:], in0=corr[:],
                                    scalar1=inv[:], scalar2=None,
                                    op0=ALU.mult)
            # ps_s4[p, mc, j] = sfac[head(mc*128+p), j]: per-partition
            # rescale factors, expanded by tiny PE matmuls against the
            # head-selector (fp32, 8 rows each — negligible PE time).
            ps_s4 = sbank.tile([P, CK, 2 * NQ], F32, tag="sbank",
                               name="ps_s4")

            def s4_stage():
                for mc in range(CK):
                    nc.tensor.matmul(ps_s4[:, mc, :], sel2_sb[:, mc, :],
                                     sfac[:], start=True, stop=True)

            # ---- phase C: v, weighting, out-projection ------------------
            # wv/wo stream in during phase A compute
            for t_sb, t_dr in ((wv_sb, wv), (wo_sb, wo)):
                for kc in range(CK):
                    nc.sync.dma_start(out=t_sb[kc][:],
                                      in_=t_dr[:, kc, :])
            for iq in range(NQ):
                xt = [xin.tile([P, TQ], BF16, tag=f"xin{kc}",
                               name=f"xtC{iq}_{kc}")
                      for kc in range(CK)]
                for kc in range(CK):
                    nc.sync.dma_start(
                        out=xt[kc][:],
                        in_=xTr[:, kc, iq * TQ:(iq + 1) * TQ])
                p2 = ppool.tile([P, CK, TQ], BF16, tag="pT")
                # software pipeline: v-matmuls for mc run 2 iterations
                # ahead of the p2 stage, covering the softmax correction
                # chain + s4 expansion at the phase B/C boundary.
                psv = {}

                def v_stage(mc, iq=iq, xt=xt, psv=psv):
                    psv[mc] = [bank.tile([P, TC], F32, tag="bank",
                                         name=f"psv_{iq}_{mc}_{t}")
                               for t in range(2)]
                    for kc in range(CK):
                        for t in range(2):
                            nc.tensor.matmul(
                                psv[mc][t][:],
                                wv_sb[kc][:, mc * P:(mc + 1) * P],
                                xt[kc][:, t * TC:(t + 1) * TC],
                                start=(kc == 0), stop=(kc == CK - 1))

                def awb_stage(mc, iq=iq, p2=p2, psv=psv):
                    # p2 = (v + bv) * s4 * exp-chunk (flash rescale folded
                    # into the per-partition scalar port)
                    v4 = qv.tile([P, 2, TC], BF16, tag="qv",
                                 name=f"v4_{iq}_{mc}")
                    for t in range(2):
                        j = iq * 2 + t
                        nc.vector.tensor_scalar(
                            out=v4[:, t, :], in0=psv[mc][t][:],
                            scalar1=bv_sb[:, mc:mc + 1],
                            scalar2=ps_s4[:, mc, j:j + 1],
                            op0=ALU.add, op1=ALU.mult)
                        nc.vector.tensor_mul(
                            p2[:, mc, t * TC:(t + 1) * TC],
                            v4[:, t, :],
                            awb_tiles[(iq, mc)][:, t * TC:(t + 1) * TC])
                    del psv[mc]
                    del awb_tiles[(iq, mc)]

                LOOKAHEAD = 2
                for mc in range(CK):
                    v_stage(mc)
                    if iq == 0 and mc == 1:
                        # s4 matmuls wait on sfac; emit them after two
                        # v-groups so they don't head-block the PE queue.
                        s4_stage()
                    if mc >= LOOKAHEAD:
                        awb_stage(mc - LOOKAHEAD)
                for mc in range(CK - LOOKAHEAD, CK):
                    awb_stage(mc)
                for nt in range(TQ // P):
                    n0 = iq * TQ + nt * P
                    for co in range(2):
                        ps_y = bank.tile([P, TC], F32, tag="bank")
                        for ci in range(CK):
                            nc.tensor.matmul(
                                ps_y[:], p2[:, ci, nt * P:(nt + 1) * P],
                                wo_sb[ci][:, co * TC:(co + 1) * TC],
                                start=(ci == 0), stop=(ci == CK - 1))
                        y_sb = ypool.tile([P, TC], F32, tag="y")
                        nc.vector.tensor_add(
                            y_sb[:], ps_y[:],
                            bo_rep[:, co * TC:(co + 1) * TC])
                        nc.sync.dma_start(
                            out=out[n0:n0 + P, co * TC:(co + 1) * TC],
                            in_=y_sb[:])
    nc.finalize()
    return nc


def _prep_core_inputs(b, x, mask, Wq, bq, Wk, bk, Wv, bv, W1x, b1, W2, b2,
                      Wo, bo, sel2, ones_r, use_mask):
    bf = ml_dtypes.bfloat16
    xT = np.ascontiguousarray(x[b].T).astype(bf)            # [C, N]
    xTr = np.ascontiguousarray(xT.reshape(CK, P, N).transpose(1, 0, 2))
    d = {
        "xTr": xTr,
        "wq": Wq, "wk": Wk, "wv": Wv, "wo": Wo,
        "w1x": W1x, "w2": W2,
        "bq": bq, "bk": bk, "bv": bv,
        "b1": b1, "b2": b2, "bo": bo,
        "sel2": sel2, "ones": ones_r,
    }
    if use_mask:
        d["mask"] = np.ascontiguousarray(
            mask[b].reshape(1, N).astype(np.int32))
    return d


def kernel(x, mask, Wq, bq, Wk, bk, Wv, bv, W1, b1, W2, b2, Wo, bo,
           trace=False):
    bf = ml_dtypes.bfloat16
    x = np.asarray(x, dtype=np.float32)
    mask = np.asarray(mask)

    def wprep(w):  # [C, C] -> [P, CK, C] bf16 (lhsT/rhs chunk layout)
        w = np.asarray(w, dtype=np.float32).astype(bf)
        return np.ascontiguousarray(w.reshape(CK, P, C).transpose(1, 0, 2))

    def bprep(v):  # [C] -> [P, CK] f32
        v = np.asarray(v, dtype=np.float32)
        return np.ascontiguousarray(v.reshape(CK, P).T)

    Wq_p, Wk_p, Wv_p, Wo_p = wprep(Wq), wprep(Wk), wprep(Wv), wprep(Wo)
    W2_p = np.asarray(W2, np.float32).astype(bf)
    bq_p, bk_p, bv_p = bprep(bq), bprep(bk), bprep(bv)
    b1_p = np.asarray(b1, np.float32).reshape(2 * D, 1)
    b2_p = np.asarray(b2, np.float32).reshape(H, 1)
    bo_p = np.asarray(bo, np.float32).astype(bf).reshape(1, C)

    # W1x[c, j] = SCALE * W1[head(c), j]: per-head reduce + MLP layer 1
    # collapsed into a single [C, 2D] contraction over channels.
    cidx = np.arange(C)
    head_of = cidx // D
    W1x = (SCALE * np.asarray(W1, np.float32))[head_of, :]      # [C, 2D]
    W1x = np.ascontiguousarray(
        W1x.reshape(CK, P, 2 * D).transpose(1, 0, 2)).astype(bf)
    sel2 = np.zeros((H, C), np.float32)
    sel2[head_of, cidx] = 1.0
    sel2 = np.ascontiguousarray(sel2.reshape(H, CK, P))
    ones_r = np.ones((1, P), np.float32).astype(bf)

    use_mask = bool(np.any(np.asarray(mask) == 0))
    nc = _build(use_mask)
    in_maps = [
        _prep_core_inputs(b, x, mask, Wq_p, bq_p, Wk_p, bk_p, Wv_p, bv_p,
                          W1x, b1_p, W2_p, b2_p, Wo_p, bo_p,
                          sel2, ones_r, use_mask)
        for b in range(B)
    ]
    res = run_bass_kernel_spmd(nc, in_maps, core_ids=list(range(B)),
                               trace=trace)
    out = np.stack([res.results[b]["out"] for b in range(B)], axis=0)
    if trace:
        kernel.last_exec_time_ns = res.exec_time_ns
        kernel.last_results = res
    return out



# revision 25
# speedup vs baseline: 1.0105x; 1.0105x over previous
"""HadamardAttention Trainium2 kernel — 8-core data-parallel over batch.

Per core (one batch element b), everything in "transposed" activation
layout [C on partitions, N on free dim]:

  phase A: qT/kT projections -> Hadamard product -> per-head reduction
           (selector matmul, SCALE folded in) -> tiny MLP -> masked
           scores awFull [H, N] (fp32)
  phase B: softmax over N (free dim) on [32, 4096]
  phase C: vT projection (x re-streamed), head-broadcast of weights
           (selector matmul), p2T = aw*vT, final out-projection which
           naturally restores natural [N, C] layout (p2T is the lhsT).

Host-side prep is layout-only (transpose/reshape) plus dtype casts to
bf16 for TensorE operands; all FLOPs happen on device.
"""
import sys

if "/opt/trn_rl_repo" not in sys.path:
    sys.path.insert(0, "/opt/trn_rl_repo")

import numpy as np
import ml_dtypes
from contextlib import ExitStack

import concourse.bass as bass
import concourse.bacc as bacc
import concourse.tile as tile
from concourse import mybir
from concourse.bass_utils import run_bass_kernel_spmd

# antenv.axon_hooks is absent in some images; shim it so trace=True can
# reach the NTFF profiler. Harmless no-op for trace=False runs.
try:
    from antenv.axon_hooks import get_axon_ntff_profile_hook  # noqa: F401
except ImportError:
    try:
        import types
        import antenv

        _hooks = types.ModuleType("antenv.axon_hooks")
        _hooks._hook = None
        _hooks.set_axon_ntff_profile_hook = lambda h: setattr(_hooks, "_hook", h)
        _hooks.get_axon_ntff_profile_hook = lambda: _hooks._hook
        sys.modules["antenv.axon_hooks"] = _hooks
        antenv.axon_hooks = _hooks
        from trn_agent_boot.trn_boot import _ntff_profile_via_ctypes

        _hooks.set_axon_ntff_profile_hook(
            _ntff_profile_via_ctypes("/opt/axon/libaxon_pjrt.so"))
    except Exception:
        pass

B, N, C, H, D = 8, 4096, 1024, 32, 32
SCALE = float(D) ** -0.5
P = 128
CK = C // P          # 8 chunks of the channel dim
NQ = 4               # token quarters
TQ = N // NQ         # 1024 tokens per quarter
TC = 512             # moving free dim per matmul
BF16 = mybir.dt.bfloat16
F32 = mybir.dt.float32
I32 = mybir.dt.int32
AF = mybir.ActivationFunctionType
ALU = mybir.AluOpType


def _build(use_mask):
    nc = bacc.Bacc("TRN2", num_devices=8)

    xTr = nc.declare_dram_parameter("xTr", [P, CK, N], BF16, isOutput=False)
    if use_mask:
        mask = nc.declare_dram_parameter("mask", [1, N], I32, isOutput=False)
    wq = nc.declare_dram_parameter("wq", [P, CK, C], BF16, isOutput=False)
    wk = nc.declare_dram_parameter("wk", [P, CK, C], BF16, isOutput=False)
    wv = nc.declare_dram_parameter("wv", [P, CK, C], BF16, isOutput=False)
    wo = nc.declare_dram_parameter("wo", [P, CK, C], BF16, isOutput=False)
    # w1x[c, j] = SCALE * W1[head(c), j]: folds the per-head reduction
    # (sel1) and the MLP first layer into one PE accumulation chain.
    w1x = nc.declare_dram_parameter("w1x", [P, CK, 2 * D], BF16, isOutput=False)
    w2 = nc.declare_dram_parameter("w2", [2 * D, H], BF16, isOutput=False)
    # fp32 head-selector for the tiny s4 rescale-expansion matmuls
    sel2 = nc.declare_dram_parameter("sel2", [H, CK, P], F32, isOutput=False)
    bq = nc.declare_dram_parameter("bq", [P, CK], F32, isOutput=False)
    bk = nc.declare_dram_parameter("bk", [P, CK], F32, isOutput=False)
    bv = nc.declare_dram_parameter("bv", [P, CK], F32, isOutput=False)
    b1 = nc.declare_dram_parameter("b1", [2 * D, 1], F32, isOutput=False)
    b2 = nc.declare_dram_parameter("b2", [H, 1], F32, isOutput=False)
    bo = nc.declare_dram_parameter("bo", [1, C], BF16, isOutput=False)
    ones = nc.declare_dram_parameter("ones", [1, P], BF16, isOutput=False)
    out = nc.declare_dram_parameter("out", [N, C], F32, isOutput=True)

    with tile.TileContext(nc) as tc:
        with ExitStack() as ctx:
            wpool = ctx.enter_context(tc.tile_pool(name="wpool", bufs=1))
            const = ctx.enter_context(tc.tile_pool(name="const", bufs=1))
            narrow = ctx.enter_context(tc.tile_pool(name="narrow", bufs=1))
            small = ctx.enter_context(tc.tile_pool(name="small", bufs=2))
            xin = ctx.enter_context(tc.tile_pool(name="xin", bufs=2))
            ppool = ctx.enter_context(tc.tile_pool(name="ppool", bufs=2))
            qv = ctx.enter_context(tc.tile_pool(name="qv", bufs=4))
            awbp = ctx.enter_context(tc.tile_pool(name="awbp", bufs=8))
            ypool = ctx.enter_context(tc.tile_pool(name="ypool", bufs=4))
            bank = ctx.enter_context(
                tc.tile_pool(name="bank", bufs=6, space="PSUM"))
            sbank = ctx.enter_context(
                tc.tile_pool(name="sbank", bufs=2, space="PSUM"))
            dpool = ctx.enter_context(
                tc.tile_pool(name="dpool", bufs=1, space="DRAM"))

            # ---- constants / weights -------------------------------------
            # per-kc tiles: dependency tracking is per-tile, so chunked
            # tiles let the first matmuls start after 2 DMAs, not 16.
            wq_sb = [wpool.tile([P, C], BF16, tag=f"wq{kc}",
                                name=f"wq{kc}") for kc in range(CK)]
            wk_sb = [wpool.tile([P, C], BF16, tag=f"wk{kc}",
                                name=f"wk{kc}") for kc in range(CK)]
            wv_sb = [wpool.tile([P, C], BF16, tag=f"wv{kc}",
                                name=f"wv{kc}") for kc in range(CK)]
            wo_sb = [wpool.tile([P, C], BF16, tag=f"wo{kc}",
                                name=f"wo{kc}") for kc in range(CK)]
            # startup-critical loads first, interleaved per-kc so the first
            # accumulation group's operands land ASAP: xt(q0) + wq on the
            # sync queue, wk in parallel on the gpsimd queue.
            xt0 = [xin.tile([P, TQ], BF16, tag=f"xin{kc}", name=f"xt0_{kc}")
                   for kc in range(CK)]
            for kc in range(CK):
                nc.sync.dma_start(out=xt0[kc][:], in_=xTr[:, kc, 0:TQ])
                nc.sync.dma_start(out=wq_sb[kc][:], in_=wq[:, kc, :])
            for kc in range(CK):
                nc.sync.dma_start(out=wk_sb[kc][:], in_=wk[:, kc, :])
            w1x_sb = const.tile([P, CK, 2 * D], BF16, tag="w1x")
            w2_sb = const.tile([2 * D, H], BF16, tag="w2")
            sel2_sb = const.tile([H, CK, P], F32, tag="sel2")
            bq_sb = const.tile([P, CK], F32, tag="bq")
            bk_sb = const.tile([P, CK], F32, tag="bk")
            bv_sb = const.tile([P, CK], F32, tag="bv")
            b1_sb = const.tile([2 * D, 1], F32, tag="b1")
            b2_sb = const.tile([H, 1], F32, tag="b2")
            bo_sb = const.tile([1, C], BF16, tag="bo")
            ones_sb = const.tile([1, P], BF16, tag="ones")
            for t_sb, t_dr in ((w1x_sb, w1x), (w2_sb, w2),
                               (sel2_sb, sel2), (bq_sb, bq), (bk_sb, bk),
                               (bv_sb, bv), (b1_sb, b1), (b2_sb, b2),
                               (bo_sb, bo), (ones_sb, ones)):
                nc.sync.dma_start(out=t_sb[:], in_=t_dr[:])

            if use_mask:
                # additive mask row: 0 where mask==1, -1e9 where mask==0.
                # mask_sb borrows an xin slot (same byte size, bf16 tiles).
                mask_sb = const.tile([1, N], I32, tag="mask")
                nc.sync.dma_start(out=mask_sb[:, :], in_=mask[:, :])
                madd = narrow.tile([1, N], BF16, tag="madd")
                nc.vector.tensor_scalar(
                    out=madd[:], in0=mask_sb[:, :],
                    scalar1=1e9, scalar2=-1e9, op0=ALU.mult, op1=ALU.add)

            # bo replicated across partitions via a step-0 DMA broadcast,
            # so the out-projection bias is a DVE add (not PE matmuls).
            bo_rep = const.tile([P, C], BF16, tag="bo_rep")
            bo_ap = bo[:, :]
            bo_bcast = bass.AP(tensor=bo_ap.tensor, offset=bo_ap.offset,
                               ap=[[0, P], list(bo_ap.ap)[1]])
            nc.gpsimd.dma_start(out=bo_rep[:], in_=bo_bcast)

            awFull = narrow.tile([H, N], F32, tag="awFull")
            awn = narrow.tile([H, N], BF16, tag="awn")
            maxP = narrow.tile([H, 2 * NQ], F32, tag="maxP")
            sumP = narrow.tile([H, 2 * NQ], F32, tag="sumP")
            negmax = narrow.tile([H, 1], F32, tag="negmax")
            sums = narrow.tile([H, 1], F32, tag="sums")
            inv = narrow.tile([H, 1], F32, tag="inv")

            # awn holds the UNNORMALIZED per-chunk exp(s - chunkmax); the
            # flash-softmax correction factor is applied per-partition in
            # phase C. This lets the head-broadcast DMAs (via a DRAM
            # scratch: step-0 partition APs need a DRAM source) run during
            # phase A, off the phase B critical path.
            awn_dr = dpool.tile([H, N], BF16, tag="awn_dr")
            awb_tiles = {}

            # ---- phase A: scores ----------------------------------------
            for iq in range(NQ):
                if iq == 0:
                    xt = xt0
                else:
                    xt = [xin.tile([P, TQ], BF16, tag=f"xin{kc}",
                                   name=f"xtA{iq}_{kc}")
                          for kc in range(CK)]
                    for kc in range(CK):
                        nc.sync.dma_start(
                            out=xt[kc][:],
                            in_=xTr[:, kc, iq * TQ:(iq + 1) * TQ])
                pT = ppool.tile([P, CK, TQ], BF16, tag="pT")
                for mc in range(CK):
                    ps_q = [bank.tile([P, TC], F32, tag="bank",
                                      name=f"psq_{iq}_{mc}_{t}")
                            for t in range(2)]
                    for kc in range(CK):
                        for t in range(2):
                            nc.tensor.matmul(
                                ps_q[t][:],
                                wq_sb[kc][:, mc * P:(mc + 1) * P],
                                xt[kc][:, t * TC:(t + 1) * TC],
                                start=(kc == 0), stop=(kc == CK - 1))
                    q_mc = qv.tile([P, 2, TC], BF16, tag="qv")
                    for t in range(2):
                        nc.scalar.activation(
                            q_mc[:, t, :], ps_q[t][:], AF.Identity,
                            bias=bq_sb[:, mc:mc + 1])
                    ps_k = [bank.tile([P, TC], F32, tag="bank",
                                      name=f"psk_{iq}_{mc}_{t}")
                            for t in range(2)]
                    for kc in range(CK):
                        for t in range(2):
                            nc.tensor.matmul(
                                ps_k[t][:],
                                wk_sb[kc][:, mc * P:(mc + 1) * P],
                                xt[kc][:, t * TC:(t + 1) * TC],
                                start=(kc == 0), stop=(kc == CK - 1))
                    for t in range(2):
                        nc.vector.scalar_tensor_tensor(
                            out=pT[:, mc, t * TC:(t + 1) * TC],
                            in0=ps_k[t][:], scalar=bk_sb[:, mc:mc + 1],
                            in1=q_mc[:, t, :], op0=ALU.add, op1=ALU.mult)
                for t in range(2):
                    j = iq * 2 + t
                    # (sel1 ∘ W1) fused: a1 = W1x^T @ pT directly from pT
                    ps_a1 = sbank.tile([2 * D, TC], F32, tag="sbank")
                    for ci in range(CK):
                        nc.tensor.matmul(
                            ps_a1[:], w1x_sb[:, ci, :],
                            pT[:, ci, t * TC:(t + 1) * TC],
                            start=(ci == 0), stop=(ci == CK - 1))
                    a1 = small.tile([2 * D, TC], BF16, tag="a1")
                    nc.scalar.activation(a1[:], ps_a1[:], AF.Relu,
                                         bias=b1_sb[:])
                    ps_aw2 = sbank.tile([2 * D, TC], F32, tag="sbank")
                    nc.tensor.matmul(ps_aw2[:H, :], w2_sb[:], a1[:],
                                     start=True, stop=not use_mask)
                    if use_mask:
                        nc.tensor.matmul(ps_aw2[:H, :], ones_sb[:1, :H],
                                         madd[:1, j * TC:(j + 1) * TC],
                                         start=False, stop=True)
                    nc.scalar.activation(
                        awFull[:, j * TC:(j + 1) * TC], ps_aw2[:H, :],
                        AF.Identity, bias=b2_sb[:])
                    # flash-style: per-chunk -max, then exp with that max;
                    # the global correction factor is folded into phase C.
                    nc.vector.reduce_max(
                        out=maxP[:, j:j + 1],
                        in_=awFull[:, j * TC:(j + 1) * TC],
                        axis=mybir.AxisListType.X, negate=True)
                    nc.scalar.activation(awn[:, j * TC:(j + 1) * TC],
                                         awFull[:, j * TC:(j + 1) * TC],
                                         AF.Exp, bias=maxP[:, j:j + 1],
                                         accum_out=sumP[:, j:j + 1])
                    nc.gpsimd.dma_start(out=awn_dr[:, j * TC:(j + 1) * TC],
                                        in_=awn[:, j * TC:(j + 1) * TC])
                # head-broadcast for this quarter: awn[h, n] -> awb[c, n]
                # for c in head h, as partition-broadcast DMAs on the
                # (idle) gpsimd queue. awbp pool backpressure paces them.
                for mc in range(CK):
                    awb_mc = awbp.tile([P, TQ], BF16, tag="awb",
                                       name=f"awb_{iq}_{mc}")
                    for h in range(4):
                        src = awn_dr[4 * mc + h:4 * mc + h + 1,
                                     iq * TQ:(iq + 1) * TQ]
                        bc = bass.AP(tensor=src.tensor, offset=src.offset,
                                     ap=[[0, 32]] + list(src.ap)[1:])
                        nc.gpsimd.dma_start(
                            out=awb_mc[h * 32:(h + 1) * 32, :], in_=bc)
                    awb_tiles[(iq, mc)] = awb_mc

            # ---- phase B: flash-softmax correction (tiny) ---------------
            # maxP holds -chunkmax; global negmax = min_j maxP[j].
            nc.vector.tensor_reduce(out=negmax[:], in_=maxP[:],
                                    axis=mybir.AxisListType.X,
                                    op=ALU.min)
            # corr[j] = exp(chunkmax_j - globalmax) = exp(-maxP_j + negmax)
            corr = narrow.tile([H, 2 * NQ], F32, tag="corr")
            nc.scalar.activation(corr[:], maxP[:], AF.Exp,
                                 bias=negmax[:], scale=-1.0)
            wsum = narrow.tile([H, 2 * NQ], F32, tag="wsum")
            nc.vector.tensor_mul(wsum[:], sumP[:], corr[:])
            nc.vector.reduce_sum(out=sums[:], in_=wsum[:],
                                 axis=mybir.AxisListType.X)
            nc.vector.reciprocal(out=inv[:], in_=sums[:])
            sfac = narrow.tile([H, 2 * NQ], F32, tag="sfac")
            nc.vector.tensor_scalar(out=sfac[:], in0=corr[:],
                                    scalar1=inv[:], scalar2=None,
                                    op0=ALU.mult)
            # ps_s4[p, mc, j] = sfac[head(mc*128+p), j]: per-partition
            # rescale factors, expanded by tiny PE matmuls against the
            # head-selector (fp32, 8 rows each — negligible PE time).
            ps_s4 = sbank.tile([P, CK, 2 * NQ], F32, tag="sbank",
                               name="ps_s4")

            def s4_stage():
                for mc in range(CK):
                    nc.tensor.matmul(ps_s4[:, mc, :], sel2_sb[:, mc, :],
                                     sfac[:], start=True, stop=True)

            # ---- phase C: v, weighting, out-projection ------------------
            # wv/wo stream in during phase A compute
            for t_sb, t_dr in ((wv_sb, wv), (wo_sb, wo)):
                for kc in range(CK):
                    nc.sync.dma_start(out=t_sb[kc][:],
                                      in_=t_dr[:, kc, :])
            for iq in range(NQ):
                xt = [xin.tile([P, TQ], BF16, tag=f"xin{kc}",
                               name=f"xtC{iq}_{kc}")
                      for kc in range(CK)]
                for kc in range(CK):
                    nc.sync.dma_start(
                        out=xt[kc][:],
                        in_=xTr[:, kc, iq * TQ:(iq + 1) * TQ])
                p2 = ppool.tile([P, CK, TQ], BF16, tag="pT")
                # software pipeline: v-matmuls for mc run 2 iterations
                # ahead of the p2 stage, covering the softmax correction
                # chain + s4 expansion at the phase B/C boundary.
                psv = {}

                def v_stage(mc, iq=iq, xt=xt, psv=psv):
                    psv[mc] = [bank.tile([P, TC], F32, tag="bank",
                                         name=f"psv_{iq}_{mc}_{t}")
                               for t in range(2)]
                    for kc in range(CK):
                        for t in range(2):
                            nc.tensor.matmul(
                                psv[mc][t][:],
                                wv_sb[kc][:, mc * P:(mc + 1) * P],
                                xt[kc][:, t * TC:(t + 1) * TC],
                                start=(kc == 0), stop=(kc == CK - 1))

                def awb_stage(mc, iq=iq, p2=p2, psv=psv):
                    # p2 = (v + bv) * s4 * exp-chunk (flash rescale folded
                    # into the per-partition scalar port)
                    v4 = qv.tile([P, 2, TC], BF16, tag="qv",
                                 name=f"v4_{iq}_{mc}")
                    for t in range(2):
                        j = iq * 2 + t
                        nc.vector.tensor_scalar(
                            out=v4[:, t, :], in0=psv[mc][t][:],
                            scalar1=bv_sb[:, mc:mc + 1],
                            scalar2=ps_s4[:, mc, j:j + 1],
                            op0=ALU.add, op1=ALU.mult)
                        nc.vector.tensor_mul(
                            p2[:, mc, t * TC:(t + 1) * TC],
                            v4[:, t, :],
                            awb_tiles[(iq, mc)][:, t * TC:(t + 1) * TC])
                    del psv[mc]
                    del awb_tiles[(iq, mc)]

                LOOKAHEAD = 2
                for mc in range(CK):
                    v_stage(mc)
                    if iq == 0 and mc == 1:
                        # s4 matmuls wait on sfac; emit them after two
                        # v-groups so they don't head-block the PE queue.
                        s4_stage()
                    if mc >= LOOKAHEAD:
                        awb_stage(mc - LOOKAHEAD)
                for mc in range(CK - LOOKAHEAD, CK):
                    awb_stage(mc)
                for nt in range(TQ // P):
                    n0 = iq * TQ + nt * P
                    for co in range(2):
                        ps_y = bank.tile([P, TC], F32, tag="bank")
                        for ci in range(CK):
                            nc.tensor.matmul(
                                ps_y[:], p2[:, ci, nt * P:(nt + 1) * P],
                                wo_sb[ci][:, co * TC:(co + 1) * TC],
                                start=(ci == 0), stop=(ci == CK - 1))
                        y_sb = ypool.tile([P, TC], F32, tag="y")
                        nc.vector.tensor_add(
                            y_sb[:], ps_y[:],
                            bo_rep[:, co * TC:(co + 1) * TC])
                        nc.sync.dma_start(
                            out=out[n0:n0 + P, co * TC:(co + 1) * TC],
                            in_=y_sb[:])
    nc.finalize()
    return nc


def _prep_core_inputs(b, x, mask, Wq, bq, Wk, bk, Wv, bv, W1x, b1, W2, b2,
                      Wo, bo, sel2, ones_r, use_mask):
    bf = ml_dtypes.bfloat16
    xT = np.ascontiguousarray(x[b].T).astype(bf)            # [C, N]
    xTr = np.ascontiguousarray(xT.reshape(CK, P, N).transpose(1, 0, 2))
    d = {
        "xTr": xTr,
        "wq": Wq, "wk": Wk, "wv": Wv, "wo": Wo,
        "w1x": W1x, "w2": W2,
        "bq": bq, "bk": bk, "bv": bv,
        "b1": b1, "b2": b2, "bo": bo,
        "sel2": sel2, "ones": ones_r,
    }
    if use_mask:
        d["mask"] = np.ascontiguousarray(
            mask[b].reshape(1, N).astype(np.int32))
    return d


def kernel(x, mask, Wq, bq, Wk, bk, Wv, bv, W1, b1, W2, b2, Wo, bo,
           trace=False):
    bf = ml_dtypes.bfloat16
    x = np.asarray(x, dtype=np.float32)
    mask = np.asarray(mask)

    def wprep(w):  # [C, C] -> [P, CK, C] bf16 (lhsT/rhs chunk layout)
        w = np.asarray(w, dtype=np.float32).astype(bf)
        return np.ascontiguousarray(w.reshape(CK, P, C).transpose(1, 0, 2))

    def bprep(v):  # [C] -> [P, CK] f32
        v = np.asarray(v, dtype=np.float32)
        return np.ascontiguousarray(v.reshape(CK, P).T)

    Wq_p, Wk_p, Wv_p, Wo_p = wprep(Wq), wprep(Wk), wprep(Wv), wprep(Wo)
    W2_p = np.asarray(W2, np.float32).astype(bf)
    bq_p, bk_p, bv_p = bprep(bq), bprep(bk), bprep(bv)
    b1_p = np.asarray(b1, np.float32).reshape(2 * D, 1)
    b2_p = np.asarray(b2, np.float32).reshape(H, 1)
    bo_p = np.asarray(bo, np.float32).astype(bf).reshape(1, C)

    # W1x[c, j] = SCALE * W1[head(c), j]: per-head reduce + MLP layer 1
    # collapsed into a single [C, 2D] contraction over channels.
    cidx = np.arange(C)
    head_of = cidx // D
    W1x = (SCALE * np.asarray(W1, np.float32))[head_of, :]      # [C, 2D]
    W1x = np.ascontiguousarray(
        W1x.reshape(CK, P, 2 * D).transpose(1, 0, 2)).astype(bf)
    sel2 = np.zeros((H, C), np.float32)
    sel2[head_of, cidx] = 1.0
    sel2 = np.ascontiguousarray(sel2.reshape(H, CK, P))
    ones_r = np.ones((1, P), np.float32).astype(bf)

    use_mask = bool(np.any(np.asarray(mask) == 0))
    nc = _build(use_mask)
    in_maps = [
        _prep_core_inputs(b, x, mask, Wq_p, bq_p, Wk_p, bk_p, Wv_p, bv_p,
                          W1x, b1_p, W2_p, b2_p, Wo_p, bo_p,
                          sel2, ones_r, use_mask)
        for b in range(B)
    ]
    res = run_bass_kernel_spmd(nc, in_maps, core_ids=list(range(B)),
                               trace=trace)
    out = np.stack([res.results[b]["out"] for b in range(B)], axis=0)
    if trace:
        kernel.last_exec_time_ns = res.exec_time_ns
        kernel.last_results = res
    return out



# revision 30
# speedup vs baseline: 1.0256x; 1.0149x over previous
"""HadamardAttention Trainium2 kernel — 8-core data-parallel over batch.

Per core (one batch element b), everything in "transposed" activation
layout [C on partitions, N on free dim]:

  phase A: qT/kT projections -> Hadamard product -> per-head reduction
           (selector matmul, SCALE folded in) -> tiny MLP -> masked
           scores awFull [H, N] (fp32)
  phase B: softmax over N (free dim) on [32, 4096]
  phase C: vT projection (x re-streamed), head-broadcast of weights
           (selector matmul), p2T = aw*vT, final out-projection which
           naturally restores natural [N, C] layout (p2T is the lhsT).

Host-side prep is layout-only (transpose/reshape) plus dtype casts to
bf16 for TensorE operands; all FLOPs happen on device.
"""
import sys

if "/opt/trn_rl_repo" not in sys.path:
    sys.path.insert(0, "/opt/trn_rl_repo")

import numpy as np
import ml_dtypes
from contextlib import ExitStack

import concourse.bass as bass
import concourse.bacc as bacc
import concourse.tile as tile
from concourse import mybir
from concourse.bass_utils import run_bass_kernel_spmd

# antenv.axon_hooks is absent in some images; shim it so trace=True can
# reach the NTFF profiler. Harmless no-op for trace=False runs.
try:
    from antenv.axon_hooks import get_axon_ntff_profile_hook  # noqa: F401
except ImportError:
    try:
        import types
        import antenv

        _hooks = types.ModuleType("antenv.axon_hooks")
        _hooks._hook = None
        _hooks.set_axon_ntff_profile_hook = lambda h: setattr(_hooks, "_hook", h)
        _hooks.get_axon_ntff_profile_hook = lambda: _hooks._hook
        sys.modules["antenv.axon_hooks"] = _hooks
        antenv.axon_hooks = _hooks
        from trn_agent_boot.trn_boot import _ntff_profile_via_ctypes

        _hooks.set_axon_ntff_profile_hook(
            _ntff_profile_via_ctypes("/opt/axon/libaxon_pjrt.so"))
    except Exception:
        pass

B, N, C, H, D = 8, 4096, 1024, 32, 32
SCALE = float(D) ** -0.5
P = 128
CK = C // P          # 8 chunks of the channel dim
NQ = 4               # token quarters
TQ = N // NQ         # 1024 tokens per quarter
TC = 512             # moving free dim per matmul
BF16 = mybir.dt.bfloat16
F32 = mybir.dt.float32
I32 = mybir.dt.int32
AF = mybir.ActivationFunctionType
ALU = mybir.AluOpType


def _build(use_mask, use_qkbias):
    nc = bacc.Bacc("TRN2", num_devices=8)

    xTr = nc.declare_dram_parameter("xTr", [P, CK, N], BF16, isOutput=False)
    if use_mask:
        mask = nc.declare_dram_parameter("mask", [1, N], I32, isOutput=False)
    wq = nc.declare_dram_parameter("wq", [P, CK, C], BF16, isOutput=False)
    wk = nc.declare_dram_parameter("wk", [P, CK, C], BF16, isOutput=False)
    wv = nc.declare_dram_parameter("wv", [P, CK, C], BF16, isOutput=False)
    wo = nc.declare_dram_parameter("wo", [P, CK, C], BF16, isOutput=False)
    w1 = nc.declare_dram_parameter("w1", [H, 2 * D], BF16, isOutput=False)
    w2 = nc.declare_dram_parameter("w2", [2 * D, H], BF16, isOutput=False)
    ident = nc.declare_dram_parameter("ident", [P, P], BF16, isOutput=False)
    # fp32 head-selector for the tiny s4 rescale-expansion matmuls
    sel2 = nc.declare_dram_parameter("sel2", [H, CK, P], F32, isOutput=False)
    bq = nc.declare_dram_parameter("bq", [1, C], F32, isOutput=False)
    bk = nc.declare_dram_parameter("bk", [1, C], F32, isOutput=False)
    bv = nc.declare_dram_parameter("bv", [P, CK], F32, isOutput=False)
    b1 = nc.declare_dram_parameter("b1", [2 * D, 1], F32, isOutput=False)
    b2 = nc.declare_dram_parameter("b2", [H, 1], F32, isOutput=False)
    bo = nc.declare_dram_parameter("bo", [1, C], BF16, isOutput=False)
    ones = nc.declare_dram_parameter("ones", [1, P], BF16, isOutput=False)
    out = nc.declare_dram_parameter("out", [N, C], F32, isOutput=True)

    with tile.TileContext(nc) as tc:
        with ExitStack() as ctx:
            wpool = ctx.enter_context(tc.tile_pool(name="wpool", bufs=1))
            const = ctx.enter_context(tc.tile_pool(name="const", bufs=1))
            narrow = ctx.enter_context(tc.tile_pool(name="narrow", bufs=1))
            small = ctx.enter_context(tc.tile_pool(name="small", bufs=2))
            xin = ctx.enter_context(tc.tile_pool(name="xin", bufs=2))
            ppool = ctx.enter_context(tc.tile_pool(name="ppool", bufs=2))
            qv = ctx.enter_context(tc.tile_pool(name="qv", bufs=4))
            pnp = ctx.enter_context(tc.tile_pool(name="pnp", bufs=3))
            snp = ctx.enter_context(tc.tile_pool(name="snp", bufs=4))
            awbp = ctx.enter_context(tc.tile_pool(name="awbp", bufs=8))
            ypool = ctx.enter_context(tc.tile_pool(name="ypool", bufs=4))
            bank = ctx.enter_context(
                tc.tile_pool(name="bank", bufs=6, space="PSUM"))
            sbank = ctx.enter_context(
                tc.tile_pool(name="sbank", bufs=2, space="PSUM"))
            dpool = ctx.enter_context(
                tc.tile_pool(name="dpool", bufs=1, space="DRAM"))

            # ---- constants / weights -------------------------------------
            # per-kc tiles: dependency tracking is per-tile, so chunked
            # tiles let the first matmuls start after 2 DMAs, not 16.
            wq_sb = [wpool.tile([P, C], BF16, tag=f"wq{kc}",
                                name=f"wq{kc}") for kc in range(CK)]
            wk_sb = [wpool.tile([P, C], BF16, tag=f"wk{kc}",
                                name=f"wk{kc}") for kc in range(CK)]
            wv_sb = [wpool.tile([P, C], BF16, tag=f"wv{kc}",
                                name=f"wv{kc}") for kc in range(CK)]
            wo_sb = [wpool.tile([P, C], BF16, tag=f"wo{kc}",
                                name=f"wo{kc}") for kc in range(CK)]
            # startup-critical loads first, interleaved per-kc so the first
            # accumulation group's operands land ASAP: xt(q0) + wq on the
            # sync queue, wk in parallel on the gpsimd queue.
            xt0 = [xin.tile([P, TQ], BF16, tag=f"xin{kc}", name=f"xt0_{kc}")
                   for kc in range(CK)]
            for kc in range(CK):
                nc.sync.dma_start(out=xt0[kc][:], in_=xTr[:, kc, 0:TQ])
                nc.sync.dma_start(out=wq_sb[kc][:], in_=wq[:, kc, :])
            for kc in range(CK):
                nc.sync.dma_start(out=wk_sb[kc][:], in_=wk[:, kc, :])
            w1_sb = const.tile([H, 2 * D], BF16, tag="w1")
            w2_sb = const.tile([2 * D, H], BF16, tag="w2")
            ident_sb = const.tile([P, P], BF16, tag="ident")
            sel2_sb = const.tile([H, CK, P], F32, tag="sel2")
            bv_sb = const.tile([P, CK], F32, tag="bv")
            b1_sb = const.tile([2 * D, 1], F32, tag="b1")
            b2_sb = const.tile([H, 1], F32, tag="b2")
            bo_sb = const.tile([1, C], BF16, tag="bo")
            ones_sb = const.tile([1, P], BF16, tag="ones")
            for t_sb, t_dr in ((w1_sb, w1), (w2_sb, w2),
                               (ident_sb, ident), (sel2_sb, sel2),
                               (bv_sb, bv), (b1_sb, b1), (b2_sb, b2),
                               (bo_sb, bo), (ones_sb, ones)):
                nc.sync.dma_start(out=t_sb[:], in_=t_dr[:])
            if use_qkbias:
                # bias rows broadcast across partitions (step-0 DRAM AP)
                bq_bc = const.tile([P, C], F32, tag="bq_bc")
                bk_bc = const.tile([P, C], F32, tag="bk_bc")
                for t_sb, t_dr in ((bq_bc, bq), (bk_bc, bk)):
                    ap = t_dr[:, :]
                    bc = bass.AP(tensor=ap.tensor, offset=ap.offset,
                                 ap=[[0, P], list(ap.ap)[1]])
                    nc.gpsimd.dma_start(out=t_sb[:], in_=bc)

            if use_mask:
                # additive mask row: 0 where mask==1, -1e9 where mask==0.
                # mask_sb borrows an xin slot (same byte size, bf16 tiles).
                mask_sb = const.tile([1, N], I32, tag="mask")
                nc.sync.dma_start(out=mask_sb[:, :], in_=mask[:, :])
                madd = narrow.tile([1, N], BF16, tag="madd")
                nc.vector.tensor_scalar(
                    out=madd[:], in0=mask_sb[:, :],
                    scalar1=1e9, scalar2=-1e9, op0=ALU.mult, op1=ALU.add)

            # bo replicated across partitions via a step-0 DMA broadcast,
            # so the out-projection bias is a DVE add (not PE matmuls).
            bo_rep = const.tile([P, C], BF16, tag="bo_rep")
            bo_ap = bo[:, :]
            bo_bcast = bass.AP(tensor=bo_ap.tensor, offset=bo_ap.offset,
                               ap=[[0, P], list(bo_ap.ap)[1]])
            nc.gpsimd.dma_start(out=bo_rep[:], in_=bo_bcast)

            awFull = narrow.tile([H, N], F32, tag="awFull")
            awn = narrow.tile([H, N], BF16, tag="awn")
            maxP = narrow.tile([H, 2 * NQ], F32, tag="maxP")
            sumP = narrow.tile([H, 2 * NQ], F32, tag="sumP")
            negmax = narrow.tile([H, 1], F32, tag="negmax")
            sums = narrow.tile([H, 1], F32, tag="sums")
            inv = narrow.tile([H, 1], F32, tag="inv")

            # awn holds the UNNORMALIZED per-chunk exp(s - chunkmax); the
            # flash-softmax correction factor is applied per-partition in
            # phase C. This lets the head-broadcast DMAs (via a DRAM
            # scratch: step-0 partition APs need a DRAM source) run during
            # phase A, off the phase B critical path.
            awn_dr = dpool.tile([H, N], BF16, tag="awn_dr")
            awb_tiles = {}

            # ---- phase A: scores ----------------------------------------
            for iq in range(NQ):
                if iq == 0:
                    xt = xt0
                else:
                    xt = [xin.tile([P, TQ], BF16, tag=f"xin{kc}",
                                   name=f"xtA{iq}_{kc}")
                          for kc in range(CK)]
                    for kc in range(CK):
                        nc.sync.dma_start(
                            out=xt[kc][:],
                            in_=xTr[:, kc, iq * TQ:(iq + 1) * TQ])
                # natural-layout scores: q/k tiles [token, channel] with
                # the x-tile as stationary operand; Hadamard + per-head
                # reduction happen on the DVE along the free dim, and a
                # cheap bf16 transpose brings scores to [H, N] for the
                # MLP + softmax. Transposes/MLP are emitted with a 2-tile
                # lag so their score dependencies never stall the PE.
                NT = TQ // P
                ps_t = {}
                s_bfs = {}

                def score_stage(nt, iq=iq, xt=xt):
                    pn = pnp.tile([P, H, D], BF16, tag="pn",
                                  name=f"pn_{iq}_{nt}")
                    for co in range(2):
                        ps_q = bank.tile([P, TC], F32, tag="bank",
                                         name=f"psq_{iq}_{nt}_{co}")
                        ps_k = bank.tile([P, TC], F32, tag="bank",
                                         name=f"psk_{iq}_{nt}_{co}")
                        for kc in range(CK):
                            lhsT = xt[kc][:, nt * P:(nt + 1) * P]
                            nc.tensor.matmul(
                                ps_q[:], lhsT,
                                wq_sb[kc][:, co * TC:(co + 1) * TC],
                                start=(kc == 0), stop=(kc == CK - 1))
                            nc.tensor.matmul(
                                ps_k[:], lhsT,
                                wk_sb[kc][:, co * TC:(co + 1) * TC],
                                start=(kc == 0), stop=(kc == CK - 1))
                        pn_co = pn[:, co * (H // 2):(co + 1) * (H // 2), :]
                        pn2d = pn_co.rearrange("p h d -> p (h d)")
                        if use_qkbias:
                            qb = qv.tile([P, 2, TC], BF16, tag="qv",
                                         name=f"qb_{iq}_{nt}_{co}")
                            nc.vector.tensor_tensor(
                                out=qb[:, 0, :], in0=ps_q[:],
                                in1=bq_bc[:, co * TC:(co + 1) * TC],
                                op=ALU.add)
                            nc.vector.tensor_tensor(
                                out=qb[:, 1, :], in0=ps_k[:],
                                in1=bk_bc[:, co * TC:(co + 1) * TC],
                                op=ALU.add)
                            nc.vector.scalar_tensor_tensor(
                                out=pn2d, in0=qb[:, 0, :], scalar=SCALE,
                                in1=qb[:, 1, :], op0=ALU.mult, op1=ALU.mult)
                        else:
                            # DVE reads at most one non-scalar PSUM input:
                            # bounce q through SBUF on the scalar engine.
                            q_sb = qv.tile([P, TC], BF16, tag="qv",
                                           name=f"qsb_{iq}_{nt}_{co}")
                            nc.scalar.activation(q_sb[:], ps_q[:], AF.Copy)
                            nc.vector.scalar_tensor_tensor(
                                out=pn2d, in0=ps_k[:], scalar=SCALE,
                                in1=q_sb[:], op0=ALU.mult, op1=ALU.mult)
                    s_nat = snp.tile([P, H], F32, tag="snat",
                                     name=f"sn_{iq}_{nt}")
                    nc.vector.reduce_sum(out=s_nat[:], in_=pn[:, :, :],
                                         axis=mybir.AxisListType.X)
                    s_bf = snp.tile([P, H], BF16, tag="sbf",
                                    name=f"sb_{iq}_{nt}")
                    nc.scalar.activation(s_bf[:], s_nat[:], AF.Copy)
                    s_bfs[nt] = s_bf

                def trans_stage(nt, iq=iq):
                    t = nt // 4
                    if nt % 4 == 0:
                        ps_t[t] = sbank.tile([H, 4, P], BF16, tag="sbank",
                                             name=f"pst_{iq}_{t}")
                    nc.tensor.transpose(ps_t[t][:, nt % 4, :],
                                        s_bfs[nt][:], ident_sb[:])
                    del s_bfs[nt]
                    if nt % 4 != 3:
                        return
                    j = iq * 2 + t
                    aw0 = small.tile([H, TC], BF16, tag="aw0")
                    nc.scalar.activation(
                        aw0[:], ps_t[t][:].rearrange("h a p -> h (a p)"),
                        AF.Copy)
                    del ps_t[t]
                    ps_a1 = sbank.tile([2 * D, TC], F32, tag="sbank")
                    nc.tensor.matmul(ps_a1[:], w1_sb[:], aw0[:],
                                     start=True, stop=True)
                    a1 = small.tile([2 * D, TC], BF16, tag="a1")
                    nc.scalar.activation(a1[:], ps_a1[:], AF.Relu,
                                         bias=b1_sb[:])
                    ps_aw2 = sbank.tile([2 * D, TC], F32, tag="sbank")
                    nc.tensor.matmul(ps_aw2[:H, :], w2_sb[:], a1[:],
                                     start=True, stop=not use_mask)
                    if use_mask:
                        nc.tensor.matmul(ps_aw2[:H, :], ones_sb[:1, :H],
                                         madd[:1, j * TC:(j + 1) * TC],
                                         start=False, stop=True)
                    nc.scalar.activation(
                        awFull[:, j * TC:(j + 1) * TC], ps_aw2[:H, :],
                        AF.Identity, bias=b2_sb[:])
                    # flash-style: per-chunk -max, then exp with that max;
                    # the global correction factor is folded into phase C.
                    nc.vector.reduce_max(
                        out=maxP[:, j:j + 1],
                        in_=awFull[:, j * TC:(j + 1) * TC],
                        axis=mybir.AxisListType.X, negate=True)
                    nc.scalar.activation(awn[:, j * TC:(j + 1) * TC],
                                         awFull[:, j * TC:(j + 1) * TC],
                                         AF.Exp, bias=maxP[:, j:j + 1],
                                         accum_out=sumP[:, j:j + 1])
                    nc.gpsimd.dma_start(out=awn_dr[:, j * TC:(j + 1) * TC],
                                        in_=awn[:, j * TC:(j + 1) * TC])
                    if t == 1:
                        # head-broadcast for this quarter: awn[h, n] ->
                        # awb[c, n] for c in head h, as partition-broadcast
                        # DMAs on the (idle) gpsimd queue; awbp pool
                        # backpressure paces them.
                        for mc in range(CK):
                            awb_mc = awbp.tile([P, TQ], BF16, tag="awb",
                                               name=f"awb_{iq}_{mc}")
                            for h in range(4):
                                sr = awn_dr[4 * mc + h:4 * mc + h + 1,
                                            iq * TQ:(iq + 1) * TQ]
                                bc = bass.AP(tensor=sr.tensor,
                                             offset=sr.offset,
                                             ap=[[0, 32]] + list(sr.ap)[1:])
                                nc.gpsimd.dma_start(
                                    out=awb_mc[h * 32:(h + 1) * 32, :],
                                    in_=bc)
                            awb_tiles[(iq, mc)] = awb_mc

                TLAG = 2
                for nt in range(NT):
                    score_stage(nt)
                    if nt >= TLAG:
                        trans_stage(nt - TLAG)
                for nt in range(NT - TLAG, NT):
                    trans_stage(nt)

            # ---- phase B: flash-softmax correction (tiny) ---------------
            # maxP holds -chunkmax; global negmax = min_j maxP[j].
            nc.vector.tensor_reduce(out=negmax[:], in_=maxP[:],
                                    axis=mybir.AxisListType.X,
                                    op=ALU.min)
            # corr[j] = exp(chunkmax_j - globalmax) = exp(-maxP_j + negmax)
            corr = narrow.tile([H, 2 * NQ], F32, tag="corr")
            nc.scalar.activation(corr[:], maxP[:], AF.Exp,
                                 bias=negmax[:], scale=-1.0)
            wsum = narrow.tile([H, 2 * NQ], F32, tag="wsum")
            nc.vector.tensor_mul(wsum[:], sumP[:], corr[:])
            nc.vector.reduce_sum(out=sums[:], in_=wsum[:],
                                 axis=mybir.AxisListType.X)
            nc.vector.reciprocal(out=inv[:], in_=sums[:])
            sfac = narrow.tile([H, 2 * NQ], F32, tag="sfac")
            nc.vector.tensor_scalar(out=sfac[:], in0=corr[:],
                                    scalar1=inv[:], scalar2=None,
                                    op0=ALU.mult)
            # ps_s4[p, mc, j] = sfac[head(mc*128+p), j]: per-partition
            # rescale factors, expanded by tiny PE matmuls against the
            # head-selector (fp32, 8 rows each — negligible PE time).
            ps_s4 = sbank.tile([P, CK, 2 * NQ], F32, tag="sbank",
                               name="ps_s4")

            def s4_stage():
                for mc in range(CK):
                    nc.tensor.matmul(ps_s4[:, mc, :], sel2_sb[:, mc, :],
                                     sfac[:], start=True, stop=True)

            # ---- phase C: v, weighting, out-projection ------------------
            # wv/wo stream in during phase A compute
            for t_sb, t_dr in ((wv_sb, wv), (wo_sb, wo)):
                for kc in range(CK):
                    nc.sync.dma_start(out=t_sb[kc][:],
                                      in_=t_dr[:, kc, :])
            for iq in range(NQ):
                xt = [xin.tile([P, TQ], BF16, tag=f"xin{kc}",
                               name=f"xtC{iq}_{kc}")
                      for kc in range(CK)]
                for kc in range(CK):
                    nc.sync.dma_start(
                        out=xt[kc][:],
                        in_=xTr[:, kc, iq * TQ:(iq + 1) * TQ])
                p2 = ppool.tile([P, CK, TQ], BF16, tag="pT")
                # software pipeline: v-matmuls for mc run 2 iterations
                # ahead of the p2 stage, covering the softmax correction
                # chain + s4 expansion at the phase B/C boundary.
                psv = {}

                def v_stage(mc, iq=iq, xt=xt, psv=psv):
                    psv[mc] = [bank.tile([P, TC], F32, tag="bank",
                                         name=f"psv_{iq}_{mc}_{t}")
                               for t in range(2)]
                    for kc in range(CK):
                        for t in range(2):
                            nc.tensor.matmul(
                                psv[mc][t][:],
                                wv_sb[kc][:, mc * P:(mc + 1) * P],
                                xt[kc][:, t * TC:(t + 1) * TC],
                                start=(kc == 0), stop=(kc == CK - 1))

                def awb_stage(mc, iq=iq, p2=p2, psv=psv):
                    # p2 = (v + bv) * s4 * exp-chunk (flash rescale folded
                    # into the per-partition scalar port)
                    v4 = qv.tile([P, 2, TC], BF16, tag="qv",
                                 name=f"v4_{iq}_{mc}")
                    for t in range(2):
                        j = iq * 2 + t
                        nc.vector.tensor_scalar(
                            out=v4[:, t, :], in0=psv[mc][t][:],
                            scalar1=bv_sb[:, mc:mc + 1],
                            scalar2=ps_s4[:, mc, j:j + 1],
                            op0=ALU.add, op1=ALU.mult)
                        nc.vector.tensor_mul(
                            p2[:, mc, t * TC:(t + 1) * TC],
                            v4[:, t, :],
                            awb_tiles[(iq, mc)][:, t * TC:(t + 1) * TC])
                    del psv[mc]
                    del awb_tiles[(iq, mc)]

                LOOKAHEAD = 2
                for mc in range(CK):
                    v_stage(mc)
                    if iq == 0 and mc == 1:
                        # s4 matmuls wait on sfac; emit them after two
                        # v-groups so they don't head-block the PE queue.
                        s4_stage()
                    if mc >= LOOKAHEAD:
                        awb_stage(mc - LOOKAHEAD)
                for mc in range(CK - LOOKAHEAD, CK):
                    awb_stage(mc)
                for nt in range(TQ // P):
                    n0 = iq * TQ + nt * P
                    for co in range(2):
                        ps_y = bank.tile([P, TC], F32, tag="bank")
                        for ci in range(CK):
                            nc.tensor.matmul(
                                ps_y[:], p2[:, ci, nt * P:(nt + 1) * P],
                                wo_sb[ci][:, co * TC:(co + 1) * TC],
                                start=(ci == 0), stop=(ci == CK - 1))
                        y_sb = ypool.tile([P, TC], F32, tag="y")
                        nc.vector.tensor_add(
                            y_sb[:], ps_y[:],
                            bo_rep[:, co * TC:(co + 1) * TC])
                        nc.sync.dma_start(
                            out=out[n0:n0 + P, co * TC:(co + 1) * TC],
                            in_=y_sb[:])
    nc.finalize()
    return nc


def _prep_core_inputs(b, x, mask, Wq, bq, Wk, bk, Wv, bv, W1, b1, W2, b2,
                      Wo, bo, sel2, ones_r, ident, use_mask):
    bf = ml_dtypes.bfloat16
    xT = np.ascontiguousarray(x[b].T).astype(bf)            # [C, N]
    xTr = np.ascontiguousarray(xT.reshape(CK, P, N).transpose(1, 0, 2))
    d = {
        "xTr": xTr,
        "wq": Wq, "wk": Wk, "wv": Wv, "wo": Wo,
        "w1": W1, "w2": W2,
        "bq": bq, "bk": bk, "bv": bv,
        "b1": b1, "b2": b2, "bo": bo,
        "sel2": sel2, "ones": ones_r, "ident": ident,
    }
    if use_mask:
        d["mask"] = np.ascontiguousarray(
            mask[b].reshape(1, N).astype(np.int32))
    return d


def kernel(x, mask, Wq, bq, Wk, bk, Wv, bv, W1, b1, W2, b2, Wo, bo,
           trace=False):
    bf = ml_dtypes.bfloat16
    x = np.asarray(x, dtype=np.float32)
    mask = np.asarray(mask)

    def wprep(w):  # [C, C] -> [P, CK, C] bf16 (lhsT/rhs chunk layout)
        w = np.asarray(w, dtype=np.float32).astype(bf)
        return np.ascontiguousarray(w.reshape(CK, P, C).transpose(1, 0, 2))

    def bprep(v):  # [C] -> [P, CK] f32
        v = np.asarray(v, dtype=np.float32)
        return np.ascontiguousarray(v.reshape(CK, P).T)

    Wq_p, Wk_p, Wv_p, Wo_p = wprep(Wq), wprep(Wk), wprep(Wv), wprep(Wo)
    W1_p = np.asarray(W1, np.float32).astype(bf)
    W2_p = np.asarray(W2, np.float32).astype(bf)
    bq_p = np.asarray(bq, np.float32).reshape(1, C)
    bk_p = np.asarray(bk, np.float32).reshape(1, C)
    bv_p = bprep(bv)
    b1_p = np.asarray(b1, np.float32).reshape(2 * D, 1)
    b2_p = np.asarray(b2, np.float32).reshape(H, 1)
    bo_p = np.asarray(bo, np.float32).astype(bf).reshape(1, C)

    cidx = np.arange(C)
    head_of = cidx // D
    sel2 = np.zeros((H, C), np.float32)
    sel2[head_of, cidx] = 1.0
    sel2 = np.ascontiguousarray(sel2.reshape(H, CK, P))
    ones_r = np.ones((1, P), np.float32).astype(bf)
    ident = np.eye(P, dtype=np.float32).astype(bf)

    use_mask = bool(np.any(np.asarray(mask) == 0))
    use_qkbias = bool(np.any(bq_p) or np.any(bk_p))
    nc = _build(use_mask, use_qkbias)
    in_maps = [
        _prep_core_inputs(b, x, mask, Wq_p, bq_p, Wk_p, bk_p, Wv_p, bv_p,
                          W1_p, b1_p, W2_p, b2_p, Wo_p, bo_p,
                          sel2, ones_r, ident, use_mask)
        for b in range(B)
    ]
    res = run_bass_kernel_spmd(nc, in_maps, core_ids=list(range(B)),
                               trace=trace)
    out = np.stack([res.results[b]["out"] for b in range(B)], axis=0)
    if trace:
        kernel.last_exec_time_ns = res.exec_time_ns
        kernel.last_results = res
    return out



# revision 32
# speedup vs baseline: 1.0290x; 1.0033x over previous
"""HadamardAttention Trainium2 kernel — 8-core data-parallel over batch.

Per core (one batch element b), everything in "transposed" activation
layout [C on partitions, N on free dim]:

  phase A: qT/kT projections -> Hadamard product -> per-head reduction
           (selector matmul, SCALE folded in) -> tiny MLP -> masked
           scores awFull [H, N] (fp32)
  phase B: softmax over N (free dim) on [32, 4096]
  phase C: vT projection (x re-streamed), head-broadcast of weights
           (selector matmul), p2T = aw*vT, final out-projection which
           naturally restores natural [N, C] layout (p2T is the lhsT).

Host-side prep is layout-only (transpose/reshape) plus dtype casts to
bf16 for TensorE operands; all FLOPs happen on device.
"""
import sys

if "/opt/trn_rl_repo" not in sys.path:
    sys.path.insert(0, "/opt/trn_rl_repo")

import numpy as np
import ml_dtypes
from contextlib import ExitStack

import concourse.bass as bass
import concourse.bacc as bacc
import concourse.tile as tile
from concourse import mybir
from concourse.bass_utils import run_bass_kernel_spmd

# antenv.axon_hooks is absent in some images; shim it so trace=True can
# reach the NTFF profiler. Harmless no-op for trace=False runs.
try:
    from antenv.axon_hooks import get_axon_ntff_profile_hook  # noqa: F401
except ImportError:
    try:
        import types
        import antenv

        _hooks = types.ModuleType("antenv.axon_hooks")
        _hooks._hook = None
        _hooks.set_axon_ntff_profile_hook = lambda h: setattr(_hooks, "_hook", h)
        _hooks.get_axon_ntff_profile_hook = lambda: _hooks._hook
        sys.modules["antenv.axon_hooks"] = _hooks
        antenv.axon_hooks = _hooks
        from trn_agent_boot.trn_boot import _ntff_profile_via_ctypes

        _hooks.set_axon_ntff_profile_hook(
            _ntff_profile_via_ctypes("/opt/axon/libaxon_pjrt.so"))
    except Exception:
        pass

B, N, C, H, D = 8, 4096, 1024, 32, 32
SCALE = float(D) ** -0.5
P = 128
CK = C // P          # 8 chunks of the channel dim
NQ = 4               # token quarters
TQ = N // NQ         # 1024 tokens per quarter
TC = 512             # moving free dim per matmul
BF16 = mybir.dt.bfloat16
F32 = mybir.dt.float32
I32 = mybir.dt.int32
AF = mybir.ActivationFunctionType
ALU = mybir.AluOpType


def _build(use_mask, use_qkbias, use_vbias, use_obias):
    nc = bacc.Bacc("TRN2", num_devices=8)

    xTr = nc.declare_dram_parameter("xTr", [P, CK, N], BF16, isOutput=False)
    if use_mask:
        mask = nc.declare_dram_parameter("mask", [1, N], I32, isOutput=False)
    wq = nc.declare_dram_parameter("wq", [P, CK, C], BF16, isOutput=False)
    wk = nc.declare_dram_parameter("wk", [P, CK, C], BF16, isOutput=False)
    wv = nc.declare_dram_parameter("wv", [P, CK, C], BF16, isOutput=False)
    wo = nc.declare_dram_parameter("wo", [P, CK, C], BF16, isOutput=False)
    w1 = nc.declare_dram_parameter("w1", [H, 2 * D], BF16, isOutput=False)
    w2 = nc.declare_dram_parameter("w2", [2 * D, H], BF16, isOutput=False)
    ident = nc.declare_dram_parameter("ident", [P, P], BF16, isOutput=False)
    # fp32 head-selector for the tiny s4 rescale-expansion matmuls
    sel2 = nc.declare_dram_parameter("sel2", [H, CK, P], F32, isOutput=False)
    bq = nc.declare_dram_parameter("bq", [1, C], F32, isOutput=False)
    bk = nc.declare_dram_parameter("bk", [1, C], F32, isOutput=False)
    bv = nc.declare_dram_parameter("bv", [P, CK], F32, isOutput=False)
    b1 = nc.declare_dram_parameter("b1", [2 * D, 1], F32, isOutput=False)
    b2 = nc.declare_dram_parameter("b2", [H, 1], F32, isOutput=False)
    bo = nc.declare_dram_parameter("bo", [1, C], BF16, isOutput=False)
    ones = nc.declare_dram_parameter("ones", [1, P], BF16, isOutput=False)
    out = nc.declare_dram_parameter("out", [N, C], F32, isOutput=True)

    with tile.TileContext(nc) as tc:
        with ExitStack() as ctx:
            wpool = ctx.enter_context(tc.tile_pool(name="wpool", bufs=1))
            const = ctx.enter_context(tc.tile_pool(name="const", bufs=1))
            narrow = ctx.enter_context(tc.tile_pool(name="narrow", bufs=1))
            small = ctx.enter_context(tc.tile_pool(name="small", bufs=2))
            xin = ctx.enter_context(tc.tile_pool(name="xin", bufs=2))
            ppool = ctx.enter_context(tc.tile_pool(name="ppool", bufs=2))
            qv = ctx.enter_context(tc.tile_pool(name="qv", bufs=4))
            pnp = ctx.enter_context(tc.tile_pool(name="pnp", bufs=3))
            snp = ctx.enter_context(tc.tile_pool(name="snp", bufs=4))
            awbp = ctx.enter_context(tc.tile_pool(name="awbp", bufs=8))
            ypool = ctx.enter_context(tc.tile_pool(name="ypool", bufs=4))
            bank = ctx.enter_context(
                tc.tile_pool(name="bank", bufs=6, space="PSUM"))
            sbank = ctx.enter_context(
                tc.tile_pool(name="sbank", bufs=2, space="PSUM"))
            dpool = ctx.enter_context(
                tc.tile_pool(name="dpool", bufs=1, space="DRAM"))

            # ---- constants / weights -------------------------------------
            # per-kc tiles: dependency tracking is per-tile, so chunked
            # tiles let the first matmuls start after 2 DMAs, not 16.
            wq_sb = [wpool.tile([P, C], BF16, tag=f"wq{kc}",
                                name=f"wq{kc}") for kc in range(CK)]
            wk_sb = [wpool.tile([P, C], BF16, tag=f"wk{kc}",
                                name=f"wk{kc}") for kc in range(CK)]
            wv_sb = [wpool.tile([P, C], BF16, tag=f"wv{kc}",
                                name=f"wv{kc}") for kc in range(CK)]
            wo_sb = [wpool.tile([P, C], BF16, tag=f"wo{kc}",
                                name=f"wo{kc}") for kc in range(CK)]
            # startup-critical loads first, interleaved per-kc so the first
            # accumulation group's operands land ASAP: xt(q0) + wq on the
            # sync queue, wk in parallel on the gpsimd queue.
            xt0 = [xin.tile([P, TQ], BF16, tag=f"xin{kc}", name=f"xt0_{kc}")
                   for kc in range(CK)]
            for kc in range(CK):
                nc.sync.dma_start(out=xt0[kc][:], in_=xTr[:, kc, 0:TQ])
                nc.sync.dma_start(out=wq_sb[kc][:], in_=wq[:, kc, :])
            for kc in range(CK):
                nc.sync.dma_start(out=wk_sb[kc][:], in_=wk[:, kc, :])
            w1_sb = const.tile([H, 2 * D], BF16, tag="w1")
            w2_sb = const.tile([2 * D, H], BF16, tag="w2")
            ident_sb = const.tile([P, P], BF16, tag="ident")
            sel2_sb = const.tile([H, CK, P], F32, tag="sel2")
            bv_sb = const.tile([P, CK], F32, tag="bv")
            b1_sb = const.tile([2 * D, 1], F32, tag="b1")
            b2_sb = const.tile([H, 1], F32, tag="b2")
            bo_sb = const.tile([1, C], BF16, tag="bo")
            ones_sb = const.tile([1, P], BF16, tag="ones")
            for t_sb, t_dr in ((w1_sb, w1), (w2_sb, w2),
                               (ident_sb, ident), (sel2_sb, sel2),
                               (bv_sb, bv), (b1_sb, b1), (b2_sb, b2),
                               (bo_sb, bo), (ones_sb, ones)):
                nc.sync.dma_start(out=t_sb[:], in_=t_dr[:])
            if use_qkbias:
                # bias rows broadcast across partitions (step-0 DRAM AP)
                bq_bc = const.tile([P, C], F32, tag="bq_bc")
                bk_bc = const.tile([P, C], F32, tag="bk_bc")
                for t_sb, t_dr in ((bq_bc, bq), (bk_bc, bk)):
                    ap = t_dr[:, :]
                    bc = bass.AP(tensor=ap.tensor, offset=ap.offset,
                                 ap=[[0, P], list(ap.ap)[1]])
                    nc.gpsimd.dma_start(out=t_sb[:], in_=bc)

            if use_mask:
                # additive mask row: 0 where mask==1, -1e9 where mask==0.
                # mask_sb borrows an xin slot (same byte size, bf16 tiles).
                mask_sb = const.tile([1, N], I32, tag="mask")
                nc.sync.dma_start(out=mask_sb[:, :], in_=mask[:, :])
                madd = narrow.tile([1, N], BF16, tag="madd")
                nc.vector.tensor_scalar(
                    out=madd[:], in0=mask_sb[:, :],
                    scalar1=1e9, scalar2=-1e9, op0=ALU.mult, op1=ALU.add)

            if use_obias:
                # bo replicated across partitions via a step-0 DMA
                # broadcast, so the out-projection bias is a DVE add.
                bo_rep = const.tile([P, C], BF16, tag="bo_rep")
                bo_ap = bo[:, :]
                bo_bcast = bass.AP(tensor=bo_ap.tensor, offset=bo_ap.offset,
                                   ap=[[0, P], list(bo_ap.ap)[1]])
                nc.gpsimd.dma_start(out=bo_rep[:], in_=bo_bcast)

            awFull = narrow.tile([H, N], F32, tag="awFull")
            awn = narrow.tile([H, N], BF16, tag="awn")
            maxP = narrow.tile([H, 2 * NQ], F32, tag="maxP")
            sumP = narrow.tile([H, 2 * NQ], F32, tag="sumP")
            negmax = narrow.tile([H, 1], F32, tag="negmax")
            sums = narrow.tile([H, 1], F32, tag="sums")
            inv = narrow.tile([H, 1], F32, tag="inv")

            # awn holds the UNNORMALIZED per-chunk exp(s - chunkmax); the
            # flash-softmax correction factor is applied per-partition in
            # phase C. This lets the head-broadcast DMAs (via a DRAM
            # scratch: step-0 partition APs need a DRAM source) run during
            # phase A, off the phase B critical path.
            awn_dr = dpool.tile([H, N], BF16, tag="awn_dr")
            awb_tiles = {}

            # ---- phase A: scores ----------------------------------------
            for iq in range(NQ):
                if iq == 0:
                    xt = xt0
                else:
                    xt = [xin.tile([P, TQ], BF16, tag=f"xin{kc}",
                                   name=f"xtA{iq}_{kc}")
                          for kc in range(CK)]
                    for kc in range(CK):
                        nc.sync.dma_start(
                            out=xt[kc][:],
                            in_=xTr[:, kc, iq * TQ:(iq + 1) * TQ])
                # natural-layout scores: q/k tiles [token, channel] with
                # the x-tile as stationary operand; Hadamard + per-head
                # reduction happen on the DVE along the free dim, and a
                # cheap bf16 transpose brings scores to [H, N] for the
                # MLP + softmax. Transposes/MLP are emitted with a 2-tile
                # lag so their score dependencies never stall the PE.
                NT = TQ // P
                ps_t = {}
                s_bfs = {}

                def score_stage(nt, iq=iq, xt=xt):
                    pn = pnp.tile([P, H, D], BF16, tag="pn",
                                  name=f"pn_{iq}_{nt}")
                    for co in range(2):
                        ps_q = bank.tile([P, TC], F32, tag="bank",
                                         name=f"psq_{iq}_{nt}_{co}")
                        ps_k = bank.tile([P, TC], F32, tag="bank",
                                         name=f"psk_{iq}_{nt}_{co}")
                        for kc in range(CK):
                            lhsT = xt[kc][:, nt * P:(nt + 1) * P]
                            nc.tensor.matmul(
                                ps_q[:], lhsT,
                                wq_sb[kc][:, co * TC:(co + 1) * TC],
                                start=(kc == 0), stop=(kc == CK - 1))
                            nc.tensor.matmul(
                                ps_k[:], lhsT,
                                wk_sb[kc][:, co * TC:(co + 1) * TC],
                                start=(kc == 0), stop=(kc == CK - 1))
                        pn_co = pn[:, co * (H // 2):(co + 1) * (H // 2), :]
                        pn2d = pn_co.rearrange("p h d -> p (h d)")
                        if use_qkbias:
                            qb = qv.tile([P, 2, TC], BF16, tag="qv",
                                         name=f"qb_{iq}_{nt}_{co}")
                            nc.vector.tensor_tensor(
                                out=qb[:, 0, :], in0=ps_q[:],
                                in1=bq_bc[:, co * TC:(co + 1) * TC],
                                op=ALU.add)
                            nc.vector.tensor_tensor(
                                out=qb[:, 1, :], in0=ps_k[:],
                                in1=bk_bc[:, co * TC:(co + 1) * TC],
                                op=ALU.add)
                            nc.vector.scalar_tensor_tensor(
                                out=pn2d, in0=qb[:, 0, :], scalar=SCALE,
                                in1=qb[:, 1, :], op0=ALU.mult, op1=ALU.mult)
                        else:
                            # DVE reads at most one non-scalar PSUM input:
                            # bounce q through SBUF on the scalar engine.
                            q_sb = qv.tile([P, TC], BF16, tag="qv",
                                           name=f"qsb_{iq}_{nt}_{co}")
                            nc.scalar.activation(q_sb[:], ps_q[:], AF.Copy)
                            nc.vector.scalar_tensor_tensor(
                                out=pn2d, in0=ps_k[:], scalar=SCALE,
                                in1=q_sb[:], op0=ALU.mult, op1=ALU.mult)
                    s_nat = snp.tile([P, H], F32, tag="snat",
                                     name=f"sn_{iq}_{nt}")
                    nc.vector.reduce_sum(out=s_nat[:], in_=pn[:, :, :],
                                         axis=mybir.AxisListType.X)
                    s_bf = snp.tile([P, H], BF16, tag="sbf",
                                    name=f"sb_{iq}_{nt}")
                    nc.scalar.activation(s_bf[:], s_nat[:], AF.Copy)
                    s_bfs[nt] = s_bf

                def trans_stage(nt, iq=iq):
                    t = nt // 4
                    if nt % 4 == 0:
                        ps_t[t] = sbank.tile([H, 4, P], BF16, tag="sbank",
                                             name=f"pst_{iq}_{t}")
                    nc.tensor.transpose(ps_t[t][:, nt % 4, :],
                                        s_bfs[nt][:], ident_sb[:])
                    del s_bfs[nt]
                    if nt % 4 != 3:
                        return
                    j = iq * 2 + t
                    aw0 = small.tile([H, TC], BF16, tag="aw0")
                    nc.scalar.activation(
                        aw0[:], ps_t[t][:].rearrange("h a p -> h (a p)"),
                        AF.Copy)
                    del ps_t[t]
                    ps_a1 = sbank.tile([2 * D, TC], F32, tag="sbank")
                    nc.tensor.matmul(ps_a1[:], w1_sb[:], aw0[:],
                                     start=True, stop=True)
                    a1 = small.tile([2 * D, TC], BF16, tag="a1")
                    nc.scalar.activation(a1[:], ps_a1[:], AF.Relu,
                                         bias=b1_sb[:])
                    ps_aw2 = sbank.tile([2 * D, TC], F32, tag="sbank")
                    nc.tensor.matmul(ps_aw2[:H, :], w2_sb[:], a1[:],
                                     start=True, stop=not use_mask)
                    if use_mask:
                        nc.tensor.matmul(ps_aw2[:H, :], ones_sb[:1, :H],
                                         madd[:1, j * TC:(j + 1) * TC],
                                         start=False, stop=True)
                    nc.scalar.activation(
                        awFull[:, j * TC:(j + 1) * TC], ps_aw2[:H, :],
                        AF.Identity, bias=b2_sb[:])
                    # flash-style: per-chunk -max, then exp with that max;
                    # the global correction factor is folded into phase C.
                    nc.vector.reduce_max(
                        out=maxP[:, j:j + 1],
                        in_=awFull[:, j * TC:(j + 1) * TC],
                        axis=mybir.AxisListType.X, negate=True)
                    nc.scalar.activation(awn[:, j * TC:(j + 1) * TC],
                                         awFull[:, j * TC:(j + 1) * TC],
                                         AF.Exp, bias=maxP[:, j:j + 1],
                                         accum_out=sumP[:, j:j + 1])
                    nc.gpsimd.dma_start(out=awn_dr[:, j * TC:(j + 1) * TC],
                                        in_=awn[:, j * TC:(j + 1) * TC])
                    if t == 1:
                        # head-broadcast for this quarter: awn[h, n] ->
                        # awb[c, n] for c in head h, as partition-broadcast
                        # DMAs on the (idle) gpsimd queue; awbp pool
                        # backpressure paces them.
                        for mc in range(CK):
                            awb_mc = awbp.tile([P, TQ], BF16, tag="awb",
                                               name=f"awb_{iq}_{mc}")
                            for h in range(4):
                                sr = awn_dr[4 * mc + h:4 * mc + h + 1,
                                            iq * TQ:(iq + 1) * TQ]
                                bc = bass.AP(tensor=sr.tensor,
                                             offset=sr.offset,
                                             ap=[[0, 32]] + list(sr.ap)[1:])
                                nc.gpsimd.dma_start(
                                    out=awb_mc[h * 32:(h + 1) * 32, :],
                                    in_=bc)
                            awb_tiles[(iq, mc)] = awb_mc

                TLAG = 2
                for nt in range(NT):
                    score_stage(nt)
                    if nt >= TLAG:
                        trans_stage(nt - TLAG)
                for nt in range(NT - TLAG, NT):
                    trans_stage(nt)

            # ---- phase B: flash-softmax correction (tiny) ---------------
            # maxP holds -chunkmax; global negmax = min_j maxP[j].
            nc.vector.tensor_reduce(out=negmax[:], in_=maxP[:],
                                    axis=mybir.AxisListType.X,
                                    op=ALU.min)
            # corr[j] = exp(chunkmax_j - globalmax) = exp(-maxP_j + negmax)
            corr = narrow.tile([H, 2 * NQ], F32, tag="corr")
            nc.scalar.activation(corr[:], maxP[:], AF.Exp,
                                 bias=negmax[:], scale=-1.0)
            wsum = narrow.tile([H, 2 * NQ], F32, tag="wsum")
            nc.vector.tensor_mul(wsum[:], sumP[:], corr[:])
            nc.vector.reduce_sum(out=sums[:], in_=wsum[:],
                                 axis=mybir.AxisListType.X)
            nc.vector.reciprocal(out=inv[:], in_=sums[:])
            sfac = narrow.tile([H, 2 * NQ], F32, tag="sfac")
            nc.vector.tensor_scalar(out=sfac[:], in0=corr[:],
                                    scalar1=inv[:], scalar2=None,
                                    op0=ALU.mult)
            # ps_s4[p, mc, j] = sfac[head(mc*128+p), j]: per-partition
            # rescale factors, expanded by tiny PE matmuls against the
            # head-selector (fp32, 8 rows each — negligible PE time).
            ps_s4 = sbank.tile([P, CK, 2 * NQ], F32, tag="sbank",
                               name="ps_s4")

            def s4_stage():
                for mc in range(CK):
                    nc.tensor.matmul(ps_s4[:, mc, :], sel2_sb[:, mc, :],
                                     sfac[:], start=True, stop=True)

            # ---- phase C: v, weighting, out-projection ------------------
            # wv/wo stream in during phase A compute
            for t_sb, t_dr in ((wv_sb, wv), (wo_sb, wo)):
                for kc in range(CK):
                    nc.sync.dma_start(out=t_sb[kc][:],
                                      in_=t_dr[:, kc, :])
            for iq in range(NQ):
                xt = [xin.tile([P, TQ], BF16, tag=f"xin{kc}",
                               name=f"xtC{iq}_{kc}")
                      for kc in range(CK)]
                for kc in range(CK):
                    nc.sync.dma_start(
                        out=xt[kc][:],
                        in_=xTr[:, kc, iq * TQ:(iq + 1) * TQ])
                p2 = ppool.tile([P, CK, TQ], BF16, tag="pT")
                # software pipeline: v-matmuls for mc run 2 iterations
                # ahead of the p2 stage, covering the softmax correction
                # chain + s4 expansion at the phase B/C boundary.
                psv = {}

                def v_stage(mc, iq=iq, xt=xt, psv=psv):
                    psv[mc] = [bank.tile([P, TC], F32, tag="bank",
                                         name=f"psv_{iq}_{mc}_{t}")
                               for t in range(2)]
                    for kc in range(CK):
                        for t in range(2):
                            nc.tensor.matmul(
                                psv[mc][t][:],
                                wv_sb[kc][:, mc * P:(mc + 1) * P],
                                xt[kc][:, t * TC:(t + 1) * TC],
                                start=(kc == 0), stop=(kc == CK - 1))

                def awb_stage(mc, iq=iq, p2=p2, psv=psv):
                    # p2 = (v + bv) * s4 * exp-chunk (flash rescale folded
                    # into the per-partition scalar port). With bv == 0
                    # this is a single fused stt: (v * s4) * exp.
                    if not use_vbias:
                        for t in range(2):
                            j = iq * 2 + t
                            nc.vector.scalar_tensor_tensor(
                                out=p2[:, mc, t * TC:(t + 1) * TC],
                                in0=psv[mc][t][:],
                                scalar=ps_s4[:, mc, j:j + 1],
                                in1=awb_tiles[(iq, mc)][:,
                                                        t * TC:(t + 1) * TC],
                                op0=ALU.mult, op1=ALU.mult)
                    else:
                        v4 = qv.tile([P, 2, TC], BF16, tag="qv",
                                     name=f"v4_{iq}_{mc}")
                        for t in range(2):
                            j = iq * 2 + t
                            nc.vector.tensor_scalar(
                                out=v4[:, t, :], in0=psv[mc][t][:],
                                scalar1=bv_sb[:, mc:mc + 1],
                                scalar2=ps_s4[:, mc, j:j + 1],
                                op0=ALU.add, op1=ALU.mult)
                            nc.vector.tensor_mul(
                                p2[:, mc, t * TC:(t + 1) * TC],
                                v4[:, t, :],
                                awb_tiles[(iq, mc)][:,
                                                    t * TC:(t + 1) * TC])
                    del psv[mc]
                    del awb_tiles[(iq, mc)]

                LOOKAHEAD = 2
                for mc in range(CK):
                    v_stage(mc)
                    if iq == 0 and mc == 1:
                        # s4 matmuls wait on sfac; emit them after two
                        # v-groups so they don't head-block the PE queue.
                        s4_stage()
                    if mc >= LOOKAHEAD:
                        awb_stage(mc - LOOKAHEAD)
                for mc in range(CK - LOOKAHEAD, CK):
                    awb_stage(mc)
                for nt in range(TQ // P):
                    n0 = iq * TQ + nt * P
                    for co in range(2):
                        ps_y = bank.tile([P, TC], F32, tag="bank")
                        for ci in range(CK):
                            nc.tensor.matmul(
                                ps_y[:], p2[:, ci, nt * P:(nt + 1) * P],
                                wo_sb[ci][:, co * TC:(co + 1) * TC],
                                start=(ci == 0), stop=(ci == CK - 1))
                        y_sb = ypool.tile([P, TC], F32, tag="y")
                        if use_obias:
                            nc.vector.tensor_add(
                                y_sb[:], ps_y[:],
                                bo_rep[:, co * TC:(co + 1) * TC])
                        else:
                            nc.vector.tensor_copy(out=y_sb[:], in_=ps_y[:])
                        nc.sync.dma_start(
                            out=out[n0:n0 + P, co * TC:(co + 1) * TC],
                            in_=y_sb[:])
    nc.finalize()
    return nc


def _prep_core_inputs(b, x, mask, Wq, bq, Wk, bk, Wv, bv, W1, b1, W2, b2,
                      Wo, bo, sel2, ones_r, ident, use_mask):
    bf = ml_dtypes.bfloat16
    xT = np.ascontiguousarray(x[b].T).astype(bf)            # [C, N]
    xTr = np.ascontiguousarray(xT.reshape(CK, P, N).transpose(1, 0, 2))
    d = {
        "xTr": xTr,
        "wq": Wq, "wk": Wk, "wv": Wv, "wo": Wo,
        "w1": W1, "w2": W2,
        "bq": bq, "bk": bk, "bv": bv,
        "b1": b1, "b2": b2, "bo": bo,
        "sel2": sel2, "ones": ones_r, "ident": ident,
    }
    if use_mask:
        d["mask"] = np.ascontiguousarray(
            mask[b].reshape(1, N).astype(np.int32))
    return d


def kernel(x, mask, Wq, bq, Wk, bk, Wv, bv, W1, b1, W2, b2, Wo, bo,
           trace=False):
    bf = ml_dtypes.bfloat16
    x = np.asarray(x, dtype=np.float32)
    mask = np.asarray(mask)

    def wprep(w):  # [C, C] -> [P, CK, C] bf16 (lhsT/rhs chunk layout)
        w = np.asarray(w, dtype=np.float32).astype(bf)
        return np.ascontiguousarray(w.reshape(CK, P, C).transpose(1, 0, 2))

    def bprep(v):  # [C] -> [P, CK] f32
        v = np.asarray(v, dtype=np.float32)
        return np.ascontiguousarray(v.reshape(CK, P).T)

    Wq_p, Wk_p, Wv_p, Wo_p = wprep(Wq), wprep(Wk), wprep(Wv), wprep(Wo)
    W1_p = np.asarray(W1, np.float32).astype(bf)
    W2_p = np.asarray(W2, np.float32).astype(bf)
    bq_p = np.asarray(bq, np.float32).reshape(1, C)
    bk_p = np.asarray(bk, np.float32).reshape(1, C)
    bv_p = bprep(bv)
    b1_p = np.asarray(b1, np.float32).reshape(2 * D, 1)
    b2_p = np.asarray(b2, np.float32).reshape(H, 1)
    bo_p = np.asarray(bo, np.float32).astype(bf).reshape(1, C)

    cidx = np.arange(C)
    head_of = cidx // D
    sel2 = np.zeros((H, C), np.float32)
    sel2[head_of, cidx] = 1.0
    sel2 = np.ascontiguousarray(sel2.reshape(H, CK, P))
    ones_r = np.ones((1, P), np.float32).astype(bf)
    ident = np.eye(P, dtype=np.float32).astype(bf)

    use_mask = bool(np.any(np.asarray(mask) == 0))
    use_qkbias = bool(np.any(bq_p) or np.any(bk_p))
    use_vbias = bool(np.any(np.asarray(bv, np.float32)))
    use_obias = bool(np.any(np.asarray(bo, np.float32)))
    nc = _build(use_mask, use_qkbias, use_vbias, use_obias)
    in_maps = [
        _prep_core_inputs(b, x, mask, Wq_p, bq_p, Wk_p, bk_p, Wv_p, bv_p,
                          W1_p, b1_p, W2_p, b2_p, Wo_p, bo_p,
                          sel2, ones_r, ident, use_mask)
        for b in range(B)
    ]
    res = run_bass_kernel_spmd(nc, in_maps, core_ids=list(range(B)),
                               trace=trace)
    out = np.stack([res.results[b]["out"] for b in range(B)], axis=0)
    if trace:
        kernel.last_exec_time_ns = res.exec_time_ns
        kernel.last_results = res
    return out



# revision 33
# speedup vs baseline: 1.0318x; 1.0028x over previous
"""HadamardAttention Trainium2 kernel — 8-core data-parallel over batch.

Per core (one batch element b), everything in "transposed" activation
layout [C on partitions, N on free dim]:

  phase A: qT/kT projections -> Hadamard product -> per-head reduction
           (selector matmul, SCALE folded in) -> tiny MLP -> masked
           scores awFull [H, N] (fp32)
  phase B: softmax over N (free dim) on [32, 4096]
  phase C: vT projection (x re-streamed), head-broadcast of weights
           (selector matmul), p2T = aw*vT, final out-projection which
           naturally restores natural [N, C] layout (p2T is the lhsT).

Host-side prep is layout-only (transpose/reshape) plus dtype casts to
bf16 for TensorE operands; all FLOPs happen on device.
"""
import sys

if "/opt/trn_rl_repo" not in sys.path:
    sys.path.insert(0, "/opt/trn_rl_repo")

import numpy as np
import ml_dtypes
from contextlib import ExitStack

import concourse.bass as bass
import concourse.bacc as bacc
import concourse.tile as tile
from concourse import mybir
from concourse.bass_utils import run_bass_kernel_spmd

# antenv.axon_hooks is absent in some images; shim it so trace=True can
# reach the NTFF profiler. Harmless no-op for trace=False runs.
try:
    from antenv.axon_hooks import get_axon_ntff_profile_hook  # noqa: F401
except ImportError:
    try:
        import types
        import antenv

        _hooks = types.ModuleType("antenv.axon_hooks")
        _hooks._hook = None
        _hooks.set_axon_ntff_profile_hook = lambda h: setattr(_hooks, "_hook", h)
        _hooks.get_axon_ntff_profile_hook = lambda: _hooks._hook
        sys.modules["antenv.axon_hooks"] = _hooks
        antenv.axon_hooks = _hooks
        from trn_agent_boot.trn_boot import _ntff_profile_via_ctypes

        _hooks.set_axon_ntff_profile_hook(
            _ntff_profile_via_ctypes("/opt/axon/libaxon_pjrt.so"))
    except Exception:
        pass

B, N, C, H, D = 8, 4096, 1024, 32, 32
SCALE = float(D) ** -0.5
P = 128
CK = C // P          # 8 chunks of the channel dim
NQ = 4               # token quarters
TQ = N // NQ         # 1024 tokens per quarter
TC = 512             # moving free dim per matmul
BF16 = mybir.dt.bfloat16
F32 = mybir.dt.float32
I32 = mybir.dt.int32
AF = mybir.ActivationFunctionType
ALU = mybir.AluOpType


def _build(use_mask, use_qkbias, use_vbias, use_obias):
    nc = bacc.Bacc("TRN2", num_devices=8)

    xTr = nc.declare_dram_parameter("xTr", [P, CK, N], BF16, isOutput=False)
    if use_mask:
        mask = nc.declare_dram_parameter("mask", [1, N], I32, isOutput=False)
    wq = nc.declare_dram_parameter("wq", [P, CK, C], BF16, isOutput=False)
    wk = nc.declare_dram_parameter("wk", [P, CK, C], BF16, isOutput=False)
    wv = nc.declare_dram_parameter("wv", [P, CK, C], BF16, isOutput=False)
    wo = nc.declare_dram_parameter("wo", [P, CK, C], BF16, isOutput=False)
    w1 = nc.declare_dram_parameter("w1", [H, 2 * D], BF16, isOutput=False)
    w2 = nc.declare_dram_parameter("w2", [2 * D, H], BF16, isOutput=False)
    ident = nc.declare_dram_parameter("ident", [P, P], BF16, isOutput=False)
    # fp32 head-selector for the tiny s4 rescale-expansion matmuls
    sel2 = nc.declare_dram_parameter("sel2", [H, CK, P], F32, isOutput=False)
    bq = nc.declare_dram_parameter("bq", [1, C], F32, isOutput=False)
    bk = nc.declare_dram_parameter("bk", [1, C], F32, isOutput=False)
    bv = nc.declare_dram_parameter("bv", [P, CK], F32, isOutput=False)
    b1 = nc.declare_dram_parameter("b1", [2 * D, 1], F32, isOutput=False)
    b2 = nc.declare_dram_parameter("b2", [H, 1], F32, isOutput=False)
    bo = nc.declare_dram_parameter("bo", [1, C], BF16, isOutput=False)
    ones = nc.declare_dram_parameter("ones", [1, P], BF16, isOutput=False)
    out = nc.declare_dram_parameter("out", [N, C], F32, isOutput=True)

    with tile.TileContext(nc) as tc:
        with ExitStack() as ctx:
            wpool = ctx.enter_context(tc.tile_pool(name="wpool", bufs=1))
            const = ctx.enter_context(tc.tile_pool(name="const", bufs=1))
            narrow = ctx.enter_context(tc.tile_pool(name="narrow", bufs=1))
            small = ctx.enter_context(tc.tile_pool(name="small", bufs=2))
            xin = ctx.enter_context(tc.tile_pool(name="xin", bufs=2))
            ppool = ctx.enter_context(tc.tile_pool(name="ppool", bufs=2))
            qv = ctx.enter_context(tc.tile_pool(name="qv", bufs=4))
            pnp = ctx.enter_context(tc.tile_pool(name="pnp", bufs=3))
            snp = ctx.enter_context(tc.tile_pool(name="snp", bufs=4))
            awbp = ctx.enter_context(tc.tile_pool(name="awbp", bufs=8))
            ypool = ctx.enter_context(tc.tile_pool(name="ypool", bufs=4))
            bank = ctx.enter_context(
                tc.tile_pool(name="bank", bufs=6, space="PSUM"))
            sbank = ctx.enter_context(
                tc.tile_pool(name="sbank", bufs=2, space="PSUM"))
            dpool = ctx.enter_context(
                tc.tile_pool(name="dpool", bufs=1, space="DRAM"))

            # ---- constants / weights -------------------------------------
            # per-kc tiles: dependency tracking is per-tile, so chunked
            # tiles let the first matmuls start after 2 DMAs, not 16.
            wq_sb = [wpool.tile([P, C], BF16, tag=f"wq{kc}",
                                name=f"wq{kc}") for kc in range(CK)]
            wk_sb = [wpool.tile([P, C], BF16, tag=f"wk{kc}",
                                name=f"wk{kc}") for kc in range(CK)]
            wv_sb = [wpool.tile([P, C], BF16, tag=f"wv{kc}",
                                name=f"wv{kc}") for kc in range(CK)]
            wo_sb = [wpool.tile([P, C], BF16, tag=f"wo{kc}",
                                name=f"wo{kc}") for kc in range(CK)]
            # startup-critical loads first, interleaved per-kc so the first
            # accumulation group's operands land ASAP: xt(q0) + wq on the
            # sync queue, wk in parallel on the gpsimd queue.
            xt0 = [xin.tile([P, TQ], BF16, tag=f"xin{kc}", name=f"xt0_{kc}")
                   for kc in range(CK)]
            for kc in range(CK):
                nc.sync.dma_start(out=xt0[kc][:], in_=xTr[:, kc, 0:TQ])
                nc.sync.dma_start(out=wq_sb[kc][:], in_=wq[:, kc, :])
                nc.gpsimd.dma_start(out=wk_sb[kc][:], in_=wk[:, kc, :])
            w1_sb = const.tile([H, 2 * D], BF16, tag="w1")
            w2_sb = const.tile([2 * D, H], BF16, tag="w2")
            ident_sb = const.tile([P, P], BF16, tag="ident")
            sel2_sb = const.tile([H, CK, P], F32, tag="sel2")
            bv_sb = const.tile([P, CK], F32, tag="bv")
            b1_sb = const.tile([2 * D, 1], F32, tag="b1")
            b2_sb = const.tile([H, 1], F32, tag="b2")
            bo_sb = const.tile([1, C], BF16, tag="bo")
            ones_sb = const.tile([1, P], BF16, tag="ones")
            for t_sb, t_dr in ((w1_sb, w1), (w2_sb, w2),
                               (ident_sb, ident), (sel2_sb, sel2),
                               (bv_sb, bv), (b1_sb, b1), (b2_sb, b2),
                               (bo_sb, bo), (ones_sb, ones)):
                nc.sync.dma_start(out=t_sb[:], in_=t_dr[:])
            if use_qkbias:
                # bias rows broadcast across partitions (step-0 DRAM AP)
                bq_bc = const.tile([P, C], F32, tag="bq_bc")
                bk_bc = const.tile([P, C], F32, tag="bk_bc")
                for t_sb, t_dr in ((bq_bc, bq), (bk_bc, bk)):
                    ap = t_dr[:, :]
                    bc = bass.AP(tensor=ap.tensor, offset=ap.offset,
                                 ap=[[0, P], list(ap.ap)[1]])
                    nc.gpsimd.dma_start(out=t_sb[:], in_=bc)

            if use_mask:
                # additive mask row: 0 where mask==1, -1e9 where mask==0.
                # mask_sb borrows an xin slot (same byte size, bf16 tiles).
                mask_sb = const.tile([1, N], I32, tag="mask")
                nc.sync.dma_start(out=mask_sb[:, :], in_=mask[:, :])
                madd = narrow.tile([1, N], BF16, tag="madd")
                nc.vector.tensor_scalar(
                    out=madd[:], in0=mask_sb[:, :],
                    scalar1=1e9, scalar2=-1e9, op0=ALU.mult, op1=ALU.add)

            if use_obias:
                # bo replicated across partitions via a step-0 DMA
                # broadcast, so the out-projection bias is a DVE add.
                bo_rep = const.tile([P, C], BF16, tag="bo_rep")
                bo_ap = bo[:, :]
                bo_bcast = bass.AP(tensor=bo_ap.tensor, offset=bo_ap.offset,
                                   ap=[[0, P], list(bo_ap.ap)[1]])
                nc.gpsimd.dma_start(out=bo_rep[:], in_=bo_bcast)

            awFull = narrow.tile([H, N], F32, tag="awFull")
            awn = narrow.tile([H, N], BF16, tag="awn")
            maxP = narrow.tile([H, 2 * NQ], F32, tag="maxP")
            sumP = narrow.tile([H, 2 * NQ], F32, tag="sumP")
            negmax = narrow.tile([H, 1], F32, tag="negmax")
            sums = narrow.tile([H, 1], F32, tag="sums")
            inv = narrow.tile([H, 1], F32, tag="inv")

            # awn holds the UNNORMALIZED per-chunk exp(s - chunkmax); the
            # flash-softmax correction factor is applied per-partition in
            # phase C. This lets the head-broadcast DMAs (via a DRAM
            # scratch: step-0 partition APs need a DRAM source) run during
            # phase A, off the phase B critical path.
            awn_dr = dpool.tile([H, N], BF16, tag="awn_dr")
            awb_tiles = {}

            # ---- phase A: scores ----------------------------------------
            for iq in range(NQ):
                if iq == 0:
                    xt = xt0
                else:
                    xt = [xin.tile([P, TQ], BF16, tag=f"xin{kc}",
                                   name=f"xtA{iq}_{kc}")
                          for kc in range(CK)]
                    for kc in range(CK):
                        nc.sync.dma_start(
                            out=xt[kc][:],
                            in_=xTr[:, kc, iq * TQ:(iq + 1) * TQ])
                # natural-layout scores: q/k tiles [token, channel] with
                # the x-tile as stationary operand; Hadamard + per-head
                # reduction happen on the DVE along the free dim, and a
                # cheap bf16 transpose brings scores to [H, N] for the
                # MLP + softmax. Transposes/MLP are emitted with a 2-tile
                # lag so their score dependencies never stall the PE.
                NT = TQ // P
                ps_t = {}
                s_bfs = {}

                def score_stage(nt, iq=iq, xt=xt):
                    pn = pnp.tile([P, H, D], BF16, tag="pn",
                                  name=f"pn_{iq}_{nt}")
                    for co in range(2):
                        ps_q = bank.tile([P, TC], F32, tag="bank",
                                         name=f"psq_{iq}_{nt}_{co}")
                        ps_k = bank.tile([P, TC], F32, tag="bank",
                                         name=f"psk_{iq}_{nt}_{co}")
                        for kc in range(CK):
                            lhsT = xt[kc][:, nt * P:(nt + 1) * P]
                            nc.tensor.matmul(
                                ps_q[:], lhsT,
                                wq_sb[kc][:, co * TC:(co + 1) * TC],
                                start=(kc == 0), stop=(kc == CK - 1))
                            nc.tensor.matmul(
                                ps_k[:], lhsT,
                                wk_sb[kc][:, co * TC:(co + 1) * TC],
                                start=(kc == 0), stop=(kc == CK - 1))
                        pn_co = pn[:, co * (H // 2):(co + 1) * (H // 2), :]
                        pn2d = pn_co.rearrange("p h d -> p (h d)")
                        if use_qkbias:
                            qb = qv.tile([P, 2, TC], BF16, tag="qv",
                                         name=f"qb_{iq}_{nt}_{co}")
                            nc.vector.tensor_tensor(
                                out=qb[:, 0, :], in0=ps_q[:],
                                in1=bq_bc[:, co * TC:(co + 1) * TC],
                                op=ALU.add)
                            nc.vector.tensor_tensor(
                                out=qb[:, 1, :], in0=ps_k[:],
                                in1=bk_bc[:, co * TC:(co + 1) * TC],
                                op=ALU.add)
                            nc.vector.scalar_tensor_tensor(
                                out=pn2d, in0=qb[:, 0, :], scalar=SCALE,
                                in1=qb[:, 1, :], op0=ALU.mult, op1=ALU.mult)
                        else:
                            # DVE reads at most one non-scalar PSUM input:
                            # bounce q through SBUF on the scalar engine.
                            q_sb = qv.tile([P, TC], BF16, tag="qv",
                                           name=f"qsb_{iq}_{nt}_{co}")
                            nc.scalar.activation(q_sb[:], ps_q[:], AF.Copy)
                            nc.vector.scalar_tensor_tensor(
                                out=pn2d, in0=ps_k[:], scalar=SCALE,
                                in1=q_sb[:], op0=ALU.mult, op1=ALU.mult)
                    s_nat = snp.tile([P, H], F32, tag="snat",
                                     name=f"sn_{iq}_{nt}")
                    nc.vector.reduce_sum(out=s_nat[:], in_=pn[:, :, :],
                                         axis=mybir.AxisListType.X)
                    s_bf = snp.tile([P, H], BF16, tag="sbf",
                                    name=f"sb_{iq}_{nt}")
                    nc.scalar.activation(s_bf[:], s_nat[:], AF.Copy)
                    s_bfs[nt] = s_bf

                def trans_stage(nt, iq=iq):
                    t = nt // 4
                    if nt % 4 == 0:
                        ps_t[t] = sbank.tile([H, 4, P], BF16, tag="sbank",
                                             name=f"pst_{iq}_{t}")
                    nc.tensor.transpose(ps_t[t][:, nt % 4, :],
                                        s_bfs[nt][:], ident_sb[:])
                    del s_bfs[nt]
                    if nt % 4 != 3:
                        return
                    j = iq * 2 + t
                    aw0 = small.tile([H, TC], BF16, tag="aw0")
                    nc.scalar.activation(
                        aw0[:], ps_t[t][:].rearrange("h a p -> h (a p)"),
                        AF.Copy)
                    del ps_t[t]
                    ps_a1 = sbank.tile([2 * D, TC], F32, tag="sbank")
                    nc.tensor.matmul(ps_a1[:], w1_sb[:], aw0[:],
                                     start=True, stop=True)
                    a1 = small.tile([2 * D, TC], BF16, tag="a1")
                    nc.scalar.activation(a1[:], ps_a1[:], AF.Relu,
                                         bias=b1_sb[:])
                    ps_aw2 = sbank.tile([2 * D, TC], F32, tag="sbank")
                    nc.tensor.matmul(ps_aw2[:H, :], w2_sb[:], a1[:],
                                     start=True, stop=not use_mask)
                    if use_mask:
                        nc.tensor.matmul(ps_aw2[:H, :], ones_sb[:1, :H],
                                         madd[:1, j * TC:(j + 1) * TC],
                                         start=False, stop=True)
                    nc.scalar.activation(
                        awFull[:, j * TC:(j + 1) * TC], ps_aw2[:H, :],
                        AF.Identity, bias=b2_sb[:])
                    # flash-style: per-chunk -max, then exp with that max;
                    # the global correction factor is folded into phase C.
                    nc.vector.reduce_max(
                        out=maxP[:, j:j + 1],
                        in_=awFull[:, j * TC:(j + 1) * TC],
                        axis=mybir.AxisListType.X, negate=True)
                    nc.scalar.activation(awn[:, j * TC:(j + 1) * TC],
                                         awFull[:, j * TC:(j + 1) * TC],
                                         AF.Exp, bias=maxP[:, j:j + 1],
                                         accum_out=sumP[:, j:j + 1])
                    nc.gpsimd.dma_start(out=awn_dr[:, j * TC:(j + 1) * TC],
                                        in_=awn[:, j * TC:(j + 1) * TC])
                    if t == 1:
                        # head-broadcast for this quarter: awn[h, n] ->
                        # awb[c, n] for c in head h, as partition-broadcast
                        # DMAs on the (idle) gpsimd queue; awbp pool
                        # backpressure paces them.
                        for mc in range(CK):
                            awb_mc = awbp.tile([P, TQ], BF16, tag="awb",
                                               name=f"awb_{iq}_{mc}")
                            for h in range(4):
                                sr = awn_dr[4 * mc + h:4 * mc + h + 1,
                                            iq * TQ:(iq + 1) * TQ]
                                bc = bass.AP(tensor=sr.tensor,
                                             offset=sr.offset,
                                             ap=[[0, 32]] + list(sr.ap)[1:])
                                nc.gpsimd.dma_start(
                                    out=awb_mc[h * 32:(h + 1) * 32, :],
                                    in_=bc)
                            awb_tiles[(iq, mc)] = awb_mc

                TLAG = 2
                for nt in range(NT):
                    score_stage(nt)
                    if nt >= TLAG:
                        trans_stage(nt - TLAG)
                for nt in range(NT - TLAG, NT):
                    trans_stage(nt)

            # ---- phase B: flash-softmax correction (tiny) ---------------
            # maxP holds -chunkmax; global negmax = min_j maxP[j].
            nc.vector.tensor_reduce(out=negmax[:], in_=maxP[:],
                                    axis=mybir.AxisListType.X,
                                    op=ALU.min)
            # corr[j] = exp(chunkmax_j - globalmax) = exp(-maxP_j + negmax)
            corr = narrow.tile([H, 2 * NQ], F32, tag="corr")
            nc.scalar.activation(corr[:], maxP[:], AF.Exp,
                                 bias=negmax[:], scale=-1.0)
            wsum = narrow.tile([H, 2 * NQ], F32, tag="wsum")
            nc.vector.tensor_mul(wsum[:], sumP[:], corr[:])
            nc.vector.reduce_sum(out=sums[:], in_=wsum[:],
                                 axis=mybir.AxisListType.X)
            nc.vector.reciprocal(out=inv[:], in_=sums[:])
            sfac = narrow.tile([H, 2 * NQ], F32, tag="sfac")
            nc.vector.tensor_scalar(out=sfac[:], in0=corr[:],
                                    scalar1=inv[:], scalar2=None,
                                    op0=ALU.mult)
            # ps_s4[p, mc, j] = sfac[head(mc*128+p), j]: per-partition
            # rescale factors, expanded by tiny PE matmuls against the
            # head-selector (fp32, 8 rows each — negligible PE time).
            ps_s4 = sbank.tile([P, CK, 2 * NQ], F32, tag="sbank",
                               name="ps_s4")

            def s4_stage():
                for mc in range(CK):
                    nc.tensor.matmul(ps_s4[:, mc, :], sel2_sb[:, mc, :],
                                     sfac[:], start=True, stop=True)

            # ---- phase C: v, weighting, out-projection ------------------
            # wv/wo stream in during phase A compute
            for t_sb, t_dr in ((wv_sb, wv), (wo_sb, wo)):
                for kc in range(CK):
                    nc.sync.dma_start(out=t_sb[kc][:],
                                      in_=t_dr[:, kc, :])
            for iq in range(NQ):
                xt = [xin.tile([P, TQ], BF16, tag=f"xin{kc}",
                               name=f"xtC{iq}_{kc}")
                      for kc in range(CK)]
                for kc in range(CK):
                    nc.sync.dma_start(
                        out=xt[kc][:],
                        in_=xTr[:, kc, iq * TQ:(iq + 1) * TQ])
                p2 = ppool.tile([P, CK, TQ], BF16, tag="pT")
                # software pipeline: v-matmuls for mc run 2 iterations
                # ahead of the p2 stage, covering the softmax correction
                # chain + s4 expansion at the phase B/C boundary.
                psv = {}

                def v_stage(mc, iq=iq, xt=xt, psv=psv):
                    psv[mc] = [bank.tile([P, TC], F32, tag="bank",
                                         name=f"psv_{iq}_{mc}_{t}")
                               for t in range(2)]
                    for kc in range(CK):
                        for t in range(2):
                            nc.tensor.matmul(
                                psv[mc][t][:],
                                wv_sb[kc][:, mc * P:(mc + 1) * P],
                                xt[kc][:, t * TC:(t + 1) * TC],
                                start=(kc == 0), stop=(kc == CK - 1))

                def awb_stage(mc, iq=iq, p2=p2, psv=psv):
                    # p2 = (v + bv) * s4 * exp-chunk (flash rescale folded
                    # into the per-partition scalar port). With bv == 0
                    # this is a single fused stt: (v * s4) * exp.
                    if not use_vbias:
                        for t in range(2):
                            j = iq * 2 + t
                            nc.vector.scalar_tensor_tensor(
                                out=p2[:, mc, t * TC:(t + 1) * TC],
                                in0=psv[mc][t][:],
                                scalar=ps_s4[:, mc, j:j + 1],
                                in1=awb_tiles[(iq, mc)][:,
                                                        t * TC:(t + 1) * TC],
                                op0=ALU.mult, op1=ALU.mult)
                    else:
                        v4 = qv.tile([P, 2, TC], BF16, tag="qv",
                                     name=f"v4_{iq}_{mc}")
                        for t in range(2):
                            j = iq * 2 + t
                            nc.vector.tensor_scalar(
                                out=v4[:, t, :], in0=psv[mc][t][:],
                                scalar1=bv_sb[:, mc:mc + 1],
                                scalar2=ps_s4[:, mc, j:j + 1],
                                op0=ALU.add, op1=ALU.mult)
                            nc.vector.tensor_mul(
                                p2[:, mc, t * TC:(t + 1) * TC],
                                v4[:, t, :],
                                awb_tiles[(iq, mc)][:,
                                                    t * TC:(t + 1) * TC])
                    del psv[mc]
                    del awb_tiles[(iq, mc)]

                LOOKAHEAD = 2
                for mc in range(CK):
                    v_stage(mc)
                    if iq == 0 and mc == 1:
                        # s4 matmuls wait on sfac; emit them after two
                        # v-groups so they don't head-block the PE queue.
                        s4_stage()
                    if mc >= LOOKAHEAD:
                        awb_stage(mc - LOOKAHEAD)
                for mc in range(CK - LOOKAHEAD, CK):
                    awb_stage(mc)
                for nt in range(TQ // P):
                    n0 = iq * TQ + nt * P
                    for co in range(2):
                        ps_y = bank.tile([P, TC], F32, tag="bank")
                        for ci in range(CK):
                            nc.tensor.matmul(
                                ps_y[:], p2[:, ci, nt * P:(nt + 1) * P],
                                wo_sb[ci][:, co * TC:(co + 1) * TC],
                                start=(ci == 0), stop=(ci == CK - 1))
                        y_sb = ypool.tile([P, TC], F32, tag="y")
                        if use_obias:
                            nc.vector.tensor_add(
                                y_sb[:], ps_y[:],
                                bo_rep[:, co * TC:(co + 1) * TC])
                        else:
                            nc.vector.tensor_copy(out=y_sb[:], in_=ps_y[:])
                        nc.sync.dma_start(
                            out=out[n0:n0 + P, co * TC:(co + 1) * TC],
                            in_=y_sb[:])
    nc.finalize()
    return nc


def _prep_core_inputs(b, x, mask, Wq, bq, Wk, bk, Wv, bv, W1, b1, W2, b2,
                      Wo, bo, sel2, ones_r, ident, use_mask):
    bf = ml_dtypes.bfloat16
    xT = np.ascontiguousarray(x[b].T).astype(bf)            # [C, N]
    xTr = np.ascontiguousarray(xT.reshape(CK, P, N).transpose(1, 0, 2))
    d = {
        "xTr": xTr,
        "wq": Wq, "wk": Wk, "wv": Wv, "wo": Wo,
        "w1": W1, "w2": W2,
        "bq": bq, "bk": bk, "bv": bv,
        "b1": b1, "b2": b2, "bo": bo,
        "sel2": sel2, "ones": ones_r, "ident": ident,
    }
    if use_mask:
        d["mask"] = np.ascontiguousarray(
            mask[b].reshape(1, N).astype(np.int32))
    return d


def kernel(x, mask, Wq, bq, Wk, bk, Wv, bv, W1, b1, W2, b2, Wo, bo,
           trace=False):
    bf = ml_dtypes.bfloat16
    x = np.asarray(x, dtype=np.float32)
    mask = np.asarray(mask)

    def wprep(w):  # [C, C] -> [P, CK, C] bf16 (lhsT/rhs chunk layout)
        w = np.asarray(w, dtype=np.float32).astype(bf)
        return np.ascontiguousarray(w.reshape(CK, P, C).transpose(1, 0, 2))

    def bprep(v):  # [C] -> [P, CK] f32
        v = np.asarray(v, dtype=np.float32)
        return np.ascontiguousarray(v.reshape(CK, P).T)

    Wq_p, Wk_p, Wv_p, Wo_p = wprep(Wq), wprep(Wk), wprep(Wv), wprep(Wo)
    W1_p = np.asarray(W1, np.float32).astype(bf)
    W2_p = np.asarray(W2, np.float32).astype(bf)
    bq_p = np.asarray(bq, np.float32).reshape(1, C)
    bk_p = np.asarray(bk, np.float32).reshape(1, C)
    bv_p = bprep(bv)
    b1_p = np.asarray(b1, np.float32).reshape(2 * D, 1)
    b2_p = np.asarray(b2, np.float32).reshape(H, 1)
    bo_p = np.asarray(bo, np.float32).astype(bf).reshape(1, C)

    cidx = np.arange(C)
    head_of = cidx // D
    sel2 = np.zeros((H, C), np.float32)
    sel2[head_of, cidx] = 1.0
    sel2 = np.ascontiguousarray(sel2.reshape(H, CK, P))
    ones_r = np.ones((1, P), np.float32).astype(bf)
    ident = np.eye(P, dtype=np.float32).astype(bf)

    use_mask = bool(np.any(np.asarray(mask) == 0))
    use_qkbias = bool(np.any(bq_p) or np.any(bk_p))
    use_vbias = bool(np.any(np.asarray(bv, np.float32)))
    use_obias = bool(np.any(np.asarray(bo, np.float32)))
    nc = _build(use_mask, use_qkbias, use_vbias, use_obias)
    in_maps = [
        _prep_core_inputs(b, x, mask, Wq_p, bq_p, Wk_p, bk_p, Wv_p, bv_p,
                          W1_p, b1_p, W2_p, b2_p, Wo_p, bo_p,
                          sel2, ones_r, ident, use_mask)
        for b in range(B)
    ]
    res = run_bass_kernel_spmd(nc, in_maps, core_ids=list(range(B)),
                               trace=trace)
    out = np.stack([res.results[b]["out"] for b in range(B)], axis=0)
    if trace:
        kernel.last_exec_time_ns = res.exec_time_ns
        kernel.last_results = res
    return out

